# revision 9
# baseline (speedup 1.0000x reference)
"""Trainium2 Bass kernel: 4-layer MoE transformer decoder (B=4,N=1024,C=1024,
H=16,D=64,HID=2048, layer0 dense GELU FFN, layers1-3 MoE E=8 top-2).

Sharding: tokens (B*N=4096) split 8 ways (512/core, core c = batch c//2 half
c%2). Weights replicated. Attention needs full-batch K/V -> one 8-core
AllGather per layer; readback uses partition-id-based dynamic DMA offsets.

Activations are kept feature-major ([C partitions, tokens free]) so every
matmul uses weights as the stationary operand. All matmuls fp32: the MoE gate
top-2 margins go down to 2.6e-6, so reduced-precision matmuls upstream of any
gate flip token routing vs the fp32 reference and blow the absmax error.
"""
import os, sys, types

sys.path.insert(0, "/opt/trn_rl_repo")
try:  # profiling hook (missing module in this image); harmless if absent
    from trn_agent_boot.trn_boot import _ntff_profile_via_ctypes
    if 'antenv.axon_hooks' not in sys.modules:
        _m = types.ModuleType('antenv.axon_hooks')
        _m.get_axon_ntff_profile_hook = (
            lambda: _ntff_profile_via_ctypes('/opt/axon/libaxon_pjrt.so'))
        sys.modules['antenv.axon_hooks'] = _m
except Exception:
    pass

import numpy as np
import concourse.bass as bass
import concourse.tile as tile
from concourse import bacc, mybir
from concourse.bass_utils import run_bass_kernel_spmd
from concourse.masks import make_identity

F32 = mybir.dt.float32
AF = mybir.ActivationFunctionType
OP = mybir.AluOpType

B, N, C = 4, 1024, 1024
H, D = 16, 64
HID = 2048
L, NDENSE = 4, 1
E, TOPK = 8, 2
NC_ = 8              # cores
T = 512              # tokens per core
P = 128
CT = C // P          # 8 c-tiles
HT = HID // P        # 16 hid-tiles
EPS = 1e-5

LAST_RESULT = None   # test.py reads exec_time_ns from here


def _build():
    nc = bacc.Bacc("TRN2", target_bir_lowering=False, debug=False,
                   num_devices=NC_)
    dp = nc.declare_dram_parameter
    xt_in = dp("xt", [P, CT, T], F32, isOutput=False)
    out_d = dp("out_t", [P, CT, T], F32, isOutput=True)
    W = {}
    for l in range(L):
        W[f"l{l}_ln1g"] = dp(f"l{l}_ln1g", [P, CT], F32, isOutput=False)
        W[f"l{l}_ln1b"] = dp(f"l{l}_ln1b", [P, CT], F32, isOutput=False)
        W[f"l{l}_ln2g"] = dp(f"l{l}_ln2g", [P, CT], F32, isOutput=False)
        W[f"l{l}_ln2b"] = dp(f"l{l}_ln2b", [P, CT], F32, isOutput=False)
        W[f"l{l}_qkvw"] = dp(f"l{l}_qkvw", [P, CT, 3 * H * D], F32, isOutput=False)
        W[f"l{l}_outw"] = dp(f"l{l}_outw", [P, CT, C], F32, isOutput=False)
        W[f"l{l}_outb"] = dp(f"l{l}_outb", [P, CT], F32, isOutput=False)
        if l < NDENSE:
            W[f"l{l}_ffw1"] = dp(f"l{l}_ffw1", [P, CT, HID], F32, isOutput=False)
            W[f"l{l}_ffb1"] = dp(f"l{l}_ffb1", [P, HT], F32, isOutput=False)
            W[f"l{l}_ffw2"] = dp(f"l{l}_ffw2", [P, HT, C], F32, isOutput=False)
            W[f"l{l}_ffb2"] = dp(f"l{l}_ffb2", [P, CT], F32, isOutput=False)
        else:
            W[f"l{l}_gwT"] = dp(f"l{l}_gwT", [P, CT, E], F32, isOutput=False)
            W[f"l{l}_exw1"] = dp(f"l{l}_exw1", [E, P, CT, HID], F32, isOutput=False)
            W[f"l{l}_exb1"] = dp(f"l{l}_exb1", [E, P, HT], F32, isOutput=False)
            W[f"l{l}_exw3"] = dp(f"l{l}_exw3", [E, P, CT, HID], F32, isOutput=False)
            W[f"l{l}_exb3"] = dp(f"l{l}_exb3", [E, P, HT], F32, isOutput=False)
            W[f"l{l}_exw2"] = dp(f"l{l}_exw2", [E, P, HT, C], F32, isOutput=False)
            W[f"l{l}_exb2r"] = dp(f"l{l}_exb2r", [E, CT, P], F32, isOutput=False)

    with tile.TileContext(nc) as tc:
        _emit(nc, tc, xt_in, out_d, W)
    nc.compile()
    return nc


def _emit(nc, tc, xt_in, out_d, W):
    from contextlib import ExitStack
    ctx = ExitStack()
    const = ctx.enter_context(tc.tile_pool(name="const", bufs=1))
    big = ctx.enter_context(tc.tile_pool(name="big", bufs=1))
    wp = ctx.enter_context(tc.tile_pool(name="wp", bufs=2))
    ptp = ctx.enter_context(tc.tile_pool(name="ptp", bufs=3))
    sm = ctx.enter_context(tc.tile_pool(name="sm", bufs=1))
    sm2 = ctx.enter_context(tc.tile_pool(name="sm2", bufs=2))
    dram = ctx.enter_context(tc.tile_pool(name="dram", bufs=1, space="DRAM"))
    ps_mm = ctx.enter_context(tc.tile_pool(name="ps_mm", bufs=2, space="PSUM"))
    ps_aux = ctx.enter_context(tc.tile_pool(name="ps_aux", bufs=2, space="PSUM"))
    ps_av = ctx.enter_context(tc.tile_pool(name="ps_av", bufs=2, space="PSUM"))
    ps_st = ctx.enter_context(tc.tile_pool(name="ps_st", bufs=2, space="PSUM"))

    ident = const.tile([P, P], F32)
    make_identity(nc, ident[:])
    ones_c = const.tile([P, 1], F32)      # column of ones (lhsT for col-sums)
    nc.vector.memset(ones_c[:], 1.0)
    ones_r = const.tile([1, P], F32)      # row of ones (lhsT for broadcasts)
    nc.vector.memset(ones_r[:], 1.0)

    # residual, feature-major [P, CT, T]
    resid = const.tile([P, CT, T], F32)
    nc.sync.dma_start(out=resid[:], in_=xt_in[:])

    pid = nc.sync.partition_id()
    pair_base = (pid // 2) * 2 * P        # AG-row offset of my batch pair

    def layer_norm(l, which, dst_tag):
        """LN over the feature (partition x ctile) axis; returns h tile."""
        g = sm.tile([P, CT], F32, tag="lng", name=f"g{l}{which}")
        b = sm.tile([P, CT], F32, tag="lnb", name=f"b{l}{which}")
        nc.sync.dma_start(out=g[:], in_=W[f"l{l}_ln{which}g"][:])
        nc.sync.dma_start(out=b[:], in_=W[f"l{l}_ln{which}b"][:])
        sq = big.tile([P, CT, T], F32, tag="tmp2m", name=f"sq{l}{which}")
        for ct in range(CT):
            nc.vector.tensor_mul(out=sq[:, ct, :], in0=resid[:, ct, :],
                                 in1=resid[:, ct, :])
        ps1 = ps_st.tile([1, T], F32, tag="st", name=f"ps1_{l}{which}")
        ps2 = ps_st.tile([1, T], F32, tag="st", name=f"ps2_{l}{which}")
        for ct in range(CT):
            nc.tensor.matmul(out=ps1[:], lhsT=ones_c[:], rhs=resid[:, ct, :],
                             start=(ct == 0), stop=(ct == CT - 1))
        for ct in range(CT):
            nc.tensor.matmul(out=ps2[:], lhsT=ones_c[:], rhs=sq[:, ct, :],
                             start=(ct == 0), stop=(ct == CT - 1))
        st = sm.tile([1, 6, T], F32, tag="stats", name=f"st{l}{which}")
        mu, msq, varp, rinv, rstd, numu = (st[:, i, :] for i in range(6))
        nc.vector.tensor_scalar_mul(out=mu, in0=ps1[:], scalar1=1.0 / C)
        nc.vector.tensor_scalar_mul(out=msq, in0=ps2[:], scalar1=1.0 / C)
        nc.vector.tensor_mul(out=varp, in0=mu, in1=mu)
        nc.vector.tensor_tensor(out=varp, in0=msq, in1=varp, op=OP.subtract)
        nc.vector.tensor_scalar_add(out=varp, in0=varp, scalar1=EPS)
        sd = st[:, 1, :]  # reuse msq slot
        nc.scalar.activation(out=sd, in_=varp, func=AF.Sqrt)
        nc.vector.reciprocal(out=rinv, in_=sd)
        # one Newton step: r = rinv*(1.5 - 0.5*varp*rinv^2)
        nc.vector.tensor_mul(out=sd, in0=rinv, in1=rinv)
        nc.vector.tensor_mul(out=sd, in0=sd, in1=varp)
        nc.vector.tensor_scalar(out=sd, in0=sd, scalar1=-0.5, scalar2=1.5,
                                op0=OP.mult, op1=OP.add)
        nc.vector.tensor_mul(out=rstd, in0=rinv, in1=sd)
        nc.vector.tensor_mul(out=numu, in0=mu, in1=rstd)
        nc.vector.tensor_scalar_mul(out=numu, in0=numu, scalar1=-1.0)
        psR = ps_aux.tile([P, T], F32, tag="aux", name=f"psR{l}{which}")
        psM = ps_aux.tile([P, T], F32, tag="aux", name=f"psM{l}{which}")
        nc.tensor.matmul(out=psR[:], lhsT=ones_r[:], rhs=rstd, start=True, stop=True)
        nc.tensor.matmul(out=psM[:], lhsT=ones_r[:], rhs=numu, start=True, stop=True)
        h = big.tile([P, CT, T], F32, tag=dst_tag, name=f"h{l}{which}")
        for ct in range(CT):
            nc.vector.tensor_tensor(out=h[:, ct, :], in0=resid[:, ct, :],
                                    in1=psR[:], op=OP.mult)
            nc.vector.tensor_tensor(out=h[:, ct, :], in0=h[:, ct, :],
                                    in1=psM[:], op=OP.add)
            nc.vector.tensor_scalar(out=h[:, ct, :], in0=h[:, ct, :],
                                    scalar1=g[:, ct:ct + 1], scalar2=b[:, ct:ct + 1],
                                    op0=OP.mult, op1=OP.add)
        return h

    def matmul_block(dst, dst_slice_fn, w_dram, h, kt, n_cols, l, nm,
                     act=None, bias=None, chunk_cols=512):
        """dst[.., n] = act(w.T @ h + bias); w_dram [P, kt, n_cols] fp32.
        kt = contraction tiles; streams weight chunks of chunk_cols."""
        for c0 in range(0, n_cols, chunk_cols):
            cw = min(chunk_cols, n_cols - c0)
            wc = wp.tile([P, kt, chunk_cols], F32, tag="w", name=f"w{nm}_{c0}")
            nc.sync.dma_start(out=wc[:, :, :cw], in_=w_dram[:, :, c0:c0 + cw])
            for ni in range(0, cw, P):
                psx = ps_mm.tile([P, T], F32, tag="mm", name=f"ps{nm}_{c0}_{ni}")
                for k in range(kt):
                    nc.tensor.matmul(out=psx[:], lhsT=wc[:, k, ni:ni + P],
                                     rhs=h[:, k, :], start=(k == 0),
                                     stop=(k == kt - 1))
                n_idx = (c0 + ni) // P
                dslice = dst_slice_fn(dst, n_idx)
                if act is not None:
                    bb = bias[:, n_idx:n_idx + 1] if bias is not None else 0.0
                    nc.scalar.activation(out=dslice, in_=psx[:], func=act, bias=bb)
                elif bias is not None:
                    nc.vector.tensor_scalar_add(out=dslice, in0=psx[:],
                                                scalar1=bias[:, n_idx:n_idx + 1])
                else:
                    nc.vector.tensor_copy(out=dslice, in_=psx[:])

    for l in range(L):
        # ---- LN1 + attention ----
        h = layer_norm(l, 1, "h")
        # qkT feature-major: q tiles 0..7 stay; k tiles 8..15 -> bounce
        qT = big.tile([P, CT, T], F32, tag="qT", name=f"qT{l}")
        kTl = big.tile([P, CT, T], F32, tag="big4a", name=f"kTl{l}")
        qkv_d = W[f"l{l}_qkvw"]
        matmul_block(qT, lambda d, n: d[:, n, :], qkv_d[:, :, 0:1024], h, CT,
                     1024, l, f"q{l}")
        matmul_block(kTl, lambda d, n: d[:, n, :], qkv_d[:, :, 1024:2048], h,
                     CT, 1024, l, f"k{l}")
        # v token-major: [tok128 x 4, 1024]
        vloc = big.tile([P, 4, 1024], F32, tag="big4b", name=f"vloc{l}")
        for nv in range(2):
            wc = wp.tile([P, CT, 512], F32, tag="w", name=f"wv{l}_{nv}")
            nc.sync.dma_start(out=wc[:],
                              in_=qkv_d[:, :, 2048 + nv * 512: 2048 + (nv + 1) * 512])
            for m in range(4):
                psv = ps_mm.tile([P, T], F32, tag="mm", name=f"psv{l}_{m}_{nv}")
                for k in range(CT):
                    nc.tensor.matmul(out=psv[:], lhsT=h[:, k, m * P:(m + 1) * P],
                                     rhs=wc[:, k, :], start=(k == 0),
                                     stop=(k == CT - 1))
                nc.vector.tensor_copy(out=vloc[:, m, nv * 512:(nv + 1) * 512],
                                      in_=psv[:])
        # bounce kT + v to DRAM, AllGather, read back my batch pair
        cin = dram.tile([P, 8192], F32, tag="cin", name=f"cin{l}")
        cout = dram.tile([NC_ * P, 8192], F32, tag="cout", name=f"cout{l}")
        nc.sync.dma_start(out=cin[:, 0:4096],
                          in_=kTl[:].rearrange("p c t -> p (c t)"))
        nc.sync.dma_start(out=cin[:, 4096:8192],
                          in_=vloc[:].rearrange("p c t -> p (c t)"))
        nc.gpsimd.collective_compute(
            "AllGather", OP.bypass, replica_groups=[list(range(NC_))],
            ins=[cin.opt()], outs=[cout.opt()])
        kTf = big.tile([P, CT, 1024], F32, tag="big4a", name=f"kTf{l}")
        vaug = big.tile([P, 8, H, 65], F32, tag="big4b", name=f"vaug{l}")
        nc.vector.memset(vaug[:, :, :, 64:65], 1.0)
        for r2 in range(2):
            src = cout[bass.ds(pair_base + r2 * P, P), :]
            nc.sync.dma_start(
                out=kTf[:, :, r2 * 512:(r2 + 1) * 512],
                in_=src[:, 0:4096].rearrange("p (c t) -> p c t", c=CT))
            nc.sync.dma_start(
                out=vaug[:, r2 * 4:(r2 + 1) * 4, :, 0:64],
                in_=src[:, 4096:8192].rearrange("p (c h d) -> p c h d", c=4, h=H))
        # attention per head; heads 2hp/2hp+1 share c-tile hp (rows 0-63/64-127)
        attT = big.tile([P, CT, T], F32, tag="tmp2m", name=f"attT{l}")
        for hd in range(H):
            hp, half = hd // 2, hd % 2
            rows = slice(half * 64, half * 64 + 64)
            psA = ps_av.tile([65, T], F32, tag="av", name=f"psA{l}_{hd}")
            for kc in range(8):
                psS = ps_aux.tile([P, T], F32, tag="aux", name=f"psS{l}_{hd}_{kc}")
                nc.tensor.matmul(out=psS[:], lhsT=kTf[rows, hp, kc * P:(kc + 1) * P],
                                 rhs=qT[rows, hp, :], start=True, stop=True)
                pt = ptp.tile([P, T], F32, tag="pt", name=f"pt{l}_{hd}_{kc}")
                nc.scalar.activation(out=pt[:], in_=psS[:], func=AF.Exp, scale=0.125)
                nc.tensor.matmul(out=psA[:], lhsT=vaug[:, kc, hd, :], rhs=pt[:],
                                 start=(kc == 0), stop=(kc == 7))
            av = sm2.tile([65, T], F32, tag="avs", name=f"av{l}_{hd}")
            nc.vector.tensor_copy(out=av[:], in_=psA[:])
            rec = sm2.tile([1, T], F32, tag="rec", name=f"rec{l}_{hd}")
            nc.vector.reciprocal(out=rec[:], in_=av[64:65, :])
            psB = ps_aux.tile([64, T], F32, tag="aux", name=f"psB{l}_{hd}")
            nc.tensor.matmul(out=psB[:], lhsT=ones_r[:, 0:64], rhs=rec[:],
                             start=True, stop=True)
            nc.vector.tensor_tensor(out=attT[rows, hp, :], in0=av[0:64, :],
                                    in1=psB[:], op=OP.mult)
        # out-projection + residual add
        outb = sm.tile([P, CT], F32, tag="lnb2", name=f"outb{l}")
        nc.sync.dma_start(out=outb[:], in_=W[f"l{l}_outb"][:])
        ow_d = W[f"l{l}_outw"]
        for c0 in (0, 512):
            wc = wp.tile([P, CT, 512], F32, tag="w", name=f"wo{l}_{c0}")
            nc.sync.dma_start(out=wc[:], in_=ow_d[:, :, c0:c0 + 512])
            for ni in range(4):
                ct = (c0 + ni * P) // P
                psx = ps_mm.tile([P, T], F32, tag="mm", name=f"pso{l}_{ct}")
                for k in range(CT):
                    nc.tensor.matmul(out=psx[:], lhsT=wc[:, k, ni * P:(ni + 1) * P],
                                     rhs=attT[:, k, :], start=(k == 0),
                                     stop=(k == CT - 1))
                tb = sm2.tile([P, T], F32, tag="projtmp", name=f"tb{l}_{ct}")
                nc.vector.tensor_scalar_add(out=tb[:], in0=psx[:],
                                            scalar1=outb[:, ct:ct + 1])
                nc.vector.tensor_tensor(out=resid[:, ct, :], in0=resid[:, ct, :],
                                        in1=tb[:], op=OP.add)

        # ---- LN2 + FFN/MoE ----
        h2 = layer_norm(l, 2, "h")
        if l < NDENSE:
            ffb1 = sm.tile([P, HT], F32, tag="lnb2", name=f"ffb1{l}")
            nc.sync.dma_start(out=ffb1[:], in_=W[f"l{l}_ffb1"][:])
            f1 = big.tile([P, HT, T], F32, tag="big4a", name=f"f1{l}")
            matmul_block(f1, lambda d, n: d[:, n, :], W[f"l{l}_ffw1"], h2, CT,
                         HID, l, f"ff1{l}", act=AF.Gelu, bias=ffb1)
            ffb2 = sm.tile([P, CT], F32, tag="lnb2", name=f"ffb2{l}")
            nc.sync.dma_start(out=ffb2[:], in_=W[f"l{l}_ffb2"][:])
            f2w = W[f"l{l}_ffw2"]
            for ct in range(CT):
                psx = ps_mm.tile([P, T], F32, tag="mm", name=f"psf2{l}_{ct}")
                for kh in range(2):
                    wc = wp.tile([P, CT, 512], F32, tag="w", name=f"wf2{l}_{ct}_{kh}")
                    nc.sync.dma_start(
                        out=wc[:, :, 0:P],
                        in_=f2w[:, kh * CT:(kh + 1) * CT, ct * P:(ct + 1) * P])
                    for k in range(CT):
                        nc.tensor.matmul(out=psx[:], lhsT=wc[:, k, 0:P],
                                         rhs=f1[:, kh * CT + k, :],
                                         start=(kh == 0 and k == 0),
                                         stop=(kh == 1 and k == CT - 1))
                tb = sm2.tile([P, T], F32, tag="projtmp", name=f"tf{l}_{ct}")
                nc.vector.tensor_scalar_add(out=tb[:], in0=psx[:],
                                            scalar1=ffb2[:, ct:ct + 1])
                nc.vector.tensor_tensor(out=resid[:, ct, :], in0=resid[:, ct, :],
                                        in1=tb[:], op=OP.add)
        else:
            _moe(nc, tc, l, W, h2, resid, ident, ones_r, big, wp, sm, sm2,
                 ps_mm, ps_aux, ps_st)

    nc.sync.dma_start(out=out_d[:], in_=resid[:])
    ctx.close()


def _moe(nc, tc, l, W, h2, resid, ident, ones_r, big, wp, sm, sm2, ps_mm, ps_aux, ps_st):
    # gate logits [E, T] feature-major
    gw = sm.tile([P, CT, E], F32, tag="gw", name=f"gw{l}")
    nc.sync.dma_start(out=gw[:], in_=W[f"l{l}_gwT"][:])
    psg = ps_st.tile([E, T], F32, tag="st", name=f"psg{l}")
    for k in range(CT):
        nc.tensor.matmul(out=psg[:], lhsT=gw[:, k, :], rhs=h2[:, k, :],
                         start=(k == 0), stop=(k == CT - 1))
    lg = sm.tile([E, T], F32, tag="lg", name=f"lg{l}")
    nc.vector.tensor_copy(out=lg[:], in_=psg[:])
    # transpose to token-major [128, 4, E]
    lgT = sm.tile([P, 4, E], F32, tag="lgT", name=f"lgT{l}")
    for j in range(4):
        pst = ps_st.tile([P, E], F32, tag="st", name=f"pst{l}_{j}")
        nc.tensor.transpose(out=pst[:], in_=lg[:, j * P:(j + 1) * P],
                            identity=ident[0:E, 0:E])
        nc.vector.tensor_copy(out=lgT[:, j, :], in_=pst[:])
    # top-2 mask + softmax (max-subtracted, matching reference)
    wk = sm.tile([P, 4, 6, E], F32, tag="wk", name=f"wk{l}")
    m1 = sm.tile([P, 4, 4], F32, tag="m1", name=f"m1{l}")
    for j in range(4):
        nc.vector.tensor_reduce(out=m1[:, j, 0:1], in_=lgT[:, j, :],
                                axis=mybir.AxisListType.X, op=OP.max)
        # eq-mask of the max, knock it out, then second max
        nc.vector.tensor_scalar(out=wk[:, j, 0, :], in0=lgT[:, j, :],
                                scalar1=m1[:, j, 0:1], scalar2=None,
                                op0=OP.is_equal)
        nc.vector.tensor_scalar_mul(out=wk[:, j, 1, :], in0=wk[:, j, 0, :],
                                    scalar1=1e30)
        nc.vector.tensor_tensor(out=wk[:, j, 1, :], in0=lgT[:, j, :],
                                in1=wk[:, j, 1, :], op=OP.subtract)
        nc.vector.tensor_reduce(out=m1[:, j, 1:2], in_=wk[:, j, 1, :],
                                axis=mybir.AxisListType.X, op=OP.max)
        nc.vector.tensor_scalar(out=wk[:, j, 2, :], in0=lgT[:, j, :],
                                scalar1=m1[:, j, 1:2], scalar2=None,
                                op0=OP.is_ge)
        # softmax exp(x - max)
        nc.vector.tensor_scalar_mul(out=m1[:, j, 2:3], in0=m1[:, j, 0:1],
                                    scalar1=-1.0)
        nc.scalar.activation(out=wk[:, j, 3, :], in_=lgT[:, j, :], func=AF.Exp,
                             bias=m1[:, j, 2:3])
        nc.vector.tensor_reduce(out=m1[:, j, 3:4], in_=wk[:, j, 3, :],
                                axis=mybir.AxisListType.X, op=OP.add)
        nc.vector.reciprocal(out=m1[:, j, 3:4], in_=m1[:, j, 3:4])
        nc.vector.tensor_mul(out=wk[:, j, 4, :], in0=wk[:, j, 3, :],
                             in1=wk[:, j, 2, :])
        nc.vector.tensor_scalar_mul(out=wk[:, j, 5, :], in0=wk[:, j, 4, :],
                                    scalar1=m1[:, j, 3:4])
    # weT [E, T] feature-major combine weights
    weT = sm.tile([E, T], F32, tag="lg2", name=f"weT{l}")
    for j in range(4):
        pst = ps_st.tile([E, P], F32, tag="st", name=f"psu{l}_{j}")
        nc.tensor.transpose(out=pst[:], in_=wk[:, j, 5, :], identity=ident[:])
        nc.vector.tensor_copy(out=weT[:, j * P:(j + 1) * P], in_=pst[:])

    moe_out = big.tile([P, CT, T], F32, tag="qT", name=f"moeout{l}")
    for e in range(E):
        b1 = sm.tile([P, HT], F32, tag="lnb2", name=f"exb1{l}_{e}")
        b3 = sm.tile([P, HT], F32, tag="lnb3", name=f"exb3{l}_{e}")
        nc.sync.dma_start(out=b1[:], in_=W[f"l{l}_exb1"][e])
        nc.sync.dma_start(out=b3[:], in_=W[f"l{l}_exb3"][e])
        g1 = big.tile([P, HT, T], F32, tag="big4a", name=f"g1_{l}_{e}")
        g3 = big.tile([P, HT, T], F32, tag="big4b", name=f"g3_{l}_{e}")
        w1d, w3d, w2d = (W[f"l{l}_exw1"][e], W[f"l{l}_exw3"][e], W[f"l{l}_exw2"][e])
        for c0 in range(0, HID, 512):
            for (wd, gg, bb, acts) in ((w1d, g1, b1, AF.Silu), (w3d, g3, b3, None)):
                wc = wp.tile([P, CT, 512], F32, tag="w", name=f"we{l}_{e}_{c0}_{acts}")
                nc.sync.dma_start(out=wc[:], in_=wd[:, :, c0:c0 + 512])
                for ni in range(4):
                    nt = (c0 + ni * P) // P
                    psx = ps_mm.tile([P, T], F32, tag="mm",
                                     name=f"pse{l}_{e}_{nt}_{acts}")
                    for k in range(CT):
                        nc.tensor.matmul(out=psx[:], lhsT=wc[:, k, ni * P:(ni + 1) * P],
                                         rhs=h2[:, k, :], start=(k == 0),
                                         stop=(k == CT - 1))
                    if acts is not None:
                        nc.scalar.activation(out=gg[:, nt, :], in_=psx[:],
                                             func=acts, bias=bb[:, nt:nt + 1])
                    else:
                        nc.vector.tensor_scalar_add(out=gg[:, nt, :], in0=psx[:],
                                                    scalar1=bb[:, nt:nt + 1])
        # ge = (silu(g1) * g3) * we_e  (we broadcast via K=1 matmul; DMA the
        # expert's row down to partition 0 first - matmul needs base part 0)
        werow = sm2.tile([1, T], F32, tag="werow", name=f"werow{l}_{e}")
        nc.sync.dma_start(out=werow[:], in_=weT[e:e + 1, :])
        psW = ps_aux.tile([P, T], F32, tag="aux", name=f"psW{l}_{e}")
        nc.tensor.matmul(out=psW[:], lhsT=ones_r[:], rhs=werow[:],
                         start=True, stop=True)
        for k in range(HT):
            nc.vector.tensor_tensor(out=g1[:, k, :], in0=g1[:, k, :],
                                    in1=g3[:, k, :], op=OP.mult)
            nc.vector.tensor_tensor(out=g1[:, k, :], in0=g1[:, k, :],
                                    in1=psW[:], op=OP.mult)
        # w2: [HID -> C], accumulate over experts in moe_out
        for ct in range(CT):
            psx = ps_mm.tile([P, T], F32, tag="mm", name=f"ps2{l}_{e}_{ct}")
            for kh in range(2):
                wc = wp.tile([P, CT, 512], F32, tag="w", name=f"w2{l}_{e}_{ct}_{kh}")
                nc.sync.dma_start(
                    out=wc[:, :, 0:P],
                    in_=w2d[:, kh * CT:(kh + 1) * CT, ct * P:(ct + 1) * P])
                for k in range(CT):
                    nc.tensor.matmul(out=psx[:], lhsT=wc[:, k, 0:P],
                                     rhs=g1[:, kh * CT + k, :],
                                     start=(kh == 0 and k == 0),
                                     stop=(kh == 1 and k == CT - 1))
            if e == 0:
                nc.vector.tensor_copy(out=moe_out[:, ct, :], in_=psx[:])
            else:
                nc.vector.tensor_tensor(out=moe_out[:, ct, :],
                                        in0=moe_out[:, ct, :], in1=psx[:],
                                        op=OP.add)
    # bias contribution: sum_e we_e * b2_e  == exb2r.T @ weT  (contraction E)
    b2r = sm.tile([E, CT, P], F32, tag="b2r", name=f"b2r{l}")
    nc.sync.dma_start(out=b2r[:], in_=W[f"l{l}_exb2r"][:])
    for ct in range(CT):
        psx = ps_aux.tile([P, T], F32, tag="aux", name=f"psb2{l}_{ct}")
        nc.tensor.matmul(out=psx[:], lhsT=b2r[:, ct, :], rhs=weT[:],
                         start=True, stop=True)
        nc.vector.tensor_tensor(out=moe_out[:, ct, :], in0=moe_out[:, ct, :],
                                in1=psx[:], op=OP.add)
        nc.vector.tensor_tensor(out=resid[:, ct, :], in0=resid[:, ct, :],
                                in1=moe_out[:, ct, :], op=OP.add)


def _stage_weights(params):
    fl = {}
    for l, p in enumerate(params):
        g = lambda k: np.ascontiguousarray(np.asarray(p[k], dtype=np.float32))
        fl[f"l{l}_ln1g"] = g("ln1_g").reshape(CT, P).T.copy()
        fl[f"l{l}_ln1b"] = g("ln1_b").reshape(CT, P).T.copy()
        fl[f"l{l}_ln2g"] = g("ln2_g").reshape(CT, P).T.copy()
        fl[f"l{l}_ln2b"] = g("ln2_b").reshape(CT, P).T.copy()
        fl[f"l{l}_qkvw"] = g("qkv_w").reshape(CT, P, 3 * H * D).transpose(1, 0, 2).copy()
        fl[f"l{l}_outw"] = g("out_w").reshape(CT, P, C).transpose(1, 0, 2).copy()
        fl[f"l{l}_outb"] = g("out_b").reshape(CT, P).T.copy()
        if "ff_w1" in p:
            fl[f"l{l}_ffw1"] = g("ff_w1").reshape(CT, P, HID).transpose(1, 0, 2).copy()
            fl[f"l{l}_ffb1"] = g("ff_b1").reshape(HT, P).T.copy()
            fl[f"l{l}_ffw2"] = g("ff_w2").reshape(HT, P, C).transpose(1, 0, 2).copy()
            fl[f"l{l}_ffb2"] = g("ff_b2").reshape(CT, P).T.copy()
        else:
            fl[f"l{l}_gwT"] = np.ascontiguousarray(g("gate_w").T).reshape(CT, P, E).transpose(1, 0, 2).copy()
            fl[f"l{l}_exw1"] = g("ex_w1").reshape(E, CT, P, HID).transpose(0, 2, 1, 3).copy()
            fl[f"l{l}_exb1"] = g("ex_b1").reshape(E, HT, P).transpose(0, 2, 1).copy()
            fl[f"l{l}_exw3"] = g("ex_w3").reshape(E, CT, P, HID).transpose(0, 2, 1, 3).copy()
            fl[f"l{l}_exb3"] = g("ex_b3").reshape(E, HT, P).transpose(0, 2, 1).copy()
            fl[f"l{l}_exw2"] = g("ex_w2").reshape(E, HID, C).reshape(E, HT, P, C).transpose(0, 2, 1, 3).copy()
            fl[f"l{l}_exb2r"] = g("ex_b2").reshape(E, CT, P).copy()
    return fl


_NC_CACHE = None


def kernel(x, params):
    global _NC_CACHE, LAST_RESULT
    x = np.asarray(x, dtype=np.float32)
    staged = _stage_weights(params)
    if _NC_CACHE is None:
        _NC_CACHE = _build()
    nc = _NC_CACHE
    xr = x.reshape(B, 2, T, C)
    in_maps = []
    for c in range(NC_):
        shard = xr[c // 2, c % 2]                      # [T, C]
        xt = np.ascontiguousarray(shard.T.reshape(CT, P, T).transpose(1, 0, 2))
        m = {"xt": xt}
        m.update(staged)
        in_maps.append(m)
    res = run_bass_kernel_spmd(nc, in_maps, list(range(NC_)),
                               trace=bool(os.environ.get("BASS_TRACE")))
    LAST_RESULT = res
    out = np.empty((B, 2, T, C), np.float32)
    for c in range(NC_):
        ot = res.results[c]["out_t"]                   # [P, CT, T]
        out[c // 2, c % 2] = ot.transpose(1, 0, 2).reshape(C, T).T
    return out.reshape(B, N, C)


# revision 11
# speedup vs baseline: 1.3420x; 1.3420x over previous
"""Trainium2 Bass kernel: 4-layer MoE transformer decoder (B=4,N=1024,C=1024,
H=16,D=64,HID=2048, layer0 dense GELU FFN, layers1-3 MoE E=8 top-2).

Sharding: tokens (B*N=4096) split 8 ways (512/core, core c = batch c//2 half
c%2). Weights replicated. Attention needs full-batch K/V -> one 8-core
AllGather per layer; readback uses partition-id-based dynamic DMA offsets.

Activations are kept feature-major ([C partitions, tokens free]) so every
matmul uses weights as the stationary operand. All matmuls fp32: the MoE gate
top-2 margins go down to 2.6e-6, so reduced-precision matmuls upstream of any
gate flip token routing vs the fp32 reference and blow the absmax error.
"""
import os, sys, types

sys.path.insert(0, "/opt/trn_rl_repo")
try:  # profiling hook (missing module in this image); harmless if absent
    from trn_agent_boot.trn_boot import _ntff_profile_via_ctypes
    if 'antenv.axon_hooks' not in sys.modules:
        _m = types.ModuleType('antenv.axon_hooks')
        _m.get_axon_ntff_profile_hook = (
            lambda: _ntff_profile_via_ctypes('/opt/axon/libaxon_pjrt.so'))
        sys.modules['antenv.axon_hooks'] = _m
except Exception:
    pass

import numpy as np
import concourse.bass as bass
import concourse.tile as tile
from concourse import bacc, mybir
from concourse.bass_utils import run_bass_kernel_spmd
from concourse.masks import make_identity

F32 = mybir.dt.float32
BF16 = mybir.dt.bfloat16
AF = mybir.ActivationFunctionType
OP = mybir.AluOpType

B, N, C = 4, 1024, 1024
H, D = 16, 64
HID = 2048
L, NDENSE = 4, 1
E, TOPK = 8, 2
NC_ = 8              # cores
T = 512              # tokens per core
P = 128
CT = C // P          # 8 c-tiles
HT = HID // P        # 16 hid-tiles
EPS = 1e-5

LAST_RESULT = None   # test.py reads exec_time_ns from here


def _build():
    nc = bacc.Bacc("TRN2", target_bir_lowering=False, debug=False,
                   num_devices=NC_)
    dp = nc.declare_dram_parameter
    xt_in = dp("xt", [P, CT, T], F32, isOutput=False)
    out_d = dp("out_t", [P, CT, T], F32, isOutput=True)
    W = {}
    for l in range(L):
        W[f"l{l}_ln1g"] = dp(f"l{l}_ln1g", [P, CT], F32, isOutput=False)
        W[f"l{l}_ln1b"] = dp(f"l{l}_ln1b", [P, CT], F32, isOutput=False)
        W[f"l{l}_ln2g"] = dp(f"l{l}_ln2g", [P, CT], F32, isOutput=False)
        W[f"l{l}_ln2b"] = dp(f"l{l}_ln2b", [P, CT], F32, isOutput=False)
        W[f"l{l}_qkvw"] = dp(f"l{l}_qkvw", [P, CT, 3 * H * D], F32, isOutput=False)
        W[f"l{l}_outw"] = dp(f"l{l}_outw", [P, CT, C], F32, isOutput=False)
        W[f"l{l}_outb"] = dp(f"l{l}_outb", [P, CT], F32, isOutput=False)
        if l < NDENSE:
            W[f"l{l}_ffw1"] = dp(f"l{l}_ffw1", [P, CT, HID], F32, isOutput=False)
            W[f"l{l}_ffb1"] = dp(f"l{l}_ffb1", [P, HT], F32, isOutput=False)
            W[f"l{l}_ffw2"] = dp(f"l{l}_ffw2", [P, HT, C], F32, isOutput=False)
            W[f"l{l}_ffb2"] = dp(f"l{l}_ffb2", [P, CT], F32, isOutput=False)
        else:
            W[f"l{l}_gwT"] = dp(f"l{l}_gwT", [P, CT, E], F32, isOutput=False)
            if l < L - 1:   # hi/lo bf16 pairs (3-term compensated matmuls)
                W[f"l{l}_exw1"] = dp(f"l{l}_exw1", [E, P, CT, 2, HID], BF16, isOutput=False)
                W[f"l{l}_exw3"] = dp(f"l{l}_exw3", [E, P, CT, 2, HID], BF16, isOutput=False)
                W[f"l{l}_exw2"] = dp(f"l{l}_exw2", [E, P, HT, 2, C], BF16, isOutput=False)
            else:           # last layer feeds no gate: plain bf16
                W[f"l{l}_exw1"] = dp(f"l{l}_exw1", [E, P, CT, HID], BF16, isOutput=False)
                W[f"l{l}_exw3"] = dp(f"l{l}_exw3", [E, P, CT, HID], BF16, isOutput=False)
                W[f"l{l}_exw2"] = dp(f"l{l}_exw2", [E, P, HT, C], BF16, isOutput=False)
            W[f"l{l}_exb1"] = dp(f"l{l}_exb1", [E, P, HT], F32, isOutput=False)
            W[f"l{l}_exb3"] = dp(f"l{l}_exb3", [E, P, HT], F32, isOutput=False)
            W[f"l{l}_exb2r"] = dp(f"l{l}_exb2r", [E, CT, P], F32, isOutput=False)

    with tile.TileContext(nc) as tc:
        _emit(nc, tc, xt_in, out_d, W)
    nc.compile()
    return nc


def _emit(nc, tc, xt_in, out_d, W):
    from contextlib import ExitStack
    ctx = ExitStack()
    const = ctx.enter_context(tc.tile_pool(name="const", bufs=1))
    big = ctx.enter_context(tc.tile_pool(name="big", bufs=1))
    wp = ctx.enter_context(tc.tile_pool(name="wp", bufs=2))
    ptp = ctx.enter_context(tc.tile_pool(name="ptp", bufs=3))
    sm = ctx.enter_context(tc.tile_pool(name="sm", bufs=1))
    sm2 = ctx.enter_context(tc.tile_pool(name="sm2", bufs=2))
    dram = ctx.enter_context(tc.tile_pool(name="dram", bufs=1, space="DRAM"))
    ps_mm = ctx.enter_context(tc.tile_pool(name="ps_mm", bufs=2, space="PSUM"))
    ps_aux = ctx.enter_context(tc.tile_pool(name="ps_aux", bufs=2, space="PSUM"))
    ps_av = ctx.enter_context(tc.tile_pool(name="ps_av", bufs=2, space="PSUM"))
    ps_st = ctx.enter_context(tc.tile_pool(name="ps_st", bufs=2, space="PSUM"))

    ident = const.tile([P, P], F32)
    make_identity(nc, ident[:])
    ones_c = const.tile([P, 1], F32)      # column of ones (lhsT for col-sums)
    nc.vector.memset(ones_c[:], 1.0)
    ones_r = const.tile([1, P], F32)      # row of ones (lhsT for broadcasts)
    nc.vector.memset(ones_r[:], 1.0)

    # residual, feature-major [P, CT, T]
    resid = const.tile([P, CT, T], F32)
    nc.sync.dma_start(out=resid[:], in_=xt_in[:])

    pid = nc.sync.partition_id()
    pair_base = (pid // 2) * 2 * P        # AG-row offset of my batch pair

    def layer_norm(l, which, dst_tag):
        """LN over the feature (partition x ctile) axis; returns h tile."""
        g = sm.tile([P, CT], F32, tag="lng", name=f"g{l}{which}")
        b = sm.tile([P, CT], F32, tag="lnb", name=f"b{l}{which}")
        nc.sync.dma_start(out=g[:], in_=W[f"l{l}_ln{which}g"][:])
        nc.sync.dma_start(out=b[:], in_=W[f"l{l}_ln{which}b"][:])
        sq = big.tile([P, CT, T], F32, tag="tmp2m", name=f"sq{l}{which}")
        for ct in range(CT):
            nc.vector.tensor_mul(out=sq[:, ct, :], in0=resid[:, ct, :],
                                 in1=resid[:, ct, :])
        ps1 = ps_st.tile([1, T], F32, tag="st", name=f"ps1_{l}{which}")
        ps2 = ps_st.tile([1, T], F32, tag="st", name=f"ps2_{l}{which}")
        for ct in range(CT):
            nc.tensor.matmul(out=ps1[:], lhsT=ones_c[:], rhs=resid[:, ct, :],
                             start=(ct == 0), stop=(ct == CT - 1))
        for ct in range(CT):
            nc.tensor.matmul(out=ps2[:], lhsT=ones_c[:], rhs=sq[:, ct, :],
                             start=(ct == 0), stop=(ct == CT - 1))
        st = sm.tile([1, 6, T], F32, tag="stats", name=f"st{l}{which}")
        mu, msq, varp, rinv, rstd, numu = (st[:, i, :] for i in range(6))
        nc.vector.tensor_scalar_mul(out=mu, in0=ps1[:], scalar1=1.0 / C)
        nc.vector.tensor_scalar_mul(out=msq, in0=ps2[:], scalar1=1.0 / C)
        nc.vector.tensor_mul(out=varp, in0=mu, in1=mu)
        nc.vector.tensor_tensor(out=varp, in0=msq, in1=varp, op=OP.subtract)
        nc.vector.tensor_scalar_add(out=varp, in0=varp, scalar1=EPS)
        sd = st[:, 1, :]  # reuse msq slot
        nc.scalar.activation(out=sd, in_=varp, func=AF.Sqrt)
        nc.vector.reciprocal(out=rinv, in_=sd)
        # one Newton step: r = rinv*(1.5 - 0.5*varp*rinv^2)
        nc.vector.tensor_mul(out=sd, in0=rinv, in1=rinv)
        nc.vector.tensor_mul(out=sd, in0=sd, in1=varp)
        nc.vector.tensor_scalar(out=sd, in0=sd, scalar1=-0.5, scalar2=1.5,
                                op0=OP.mult, op1=OP.add)
        nc.vector.tensor_mul(out=rstd, in0=rinv, in1=sd)
        nc.vector.tensor_mul(out=numu, in0=mu, in1=rstd)
        nc.vector.tensor_scalar_mul(out=numu, in0=numu, scalar1=-1.0)
        psR = ps_aux.tile([P, T], F32, tag="aux", name=f"psR{l}{which}")
        psM = ps_aux.tile([P, T], F32, tag="aux", name=f"psM{l}{which}")
        nc.tensor.matmul(out=psR[:], lhsT=ones_r[:], rhs=rstd, start=True, stop=True)
        nc.tensor.matmul(out=psM[:], lhsT=ones_r[:], rhs=numu, start=True, stop=True)
        h = big.tile([P, CT, T], F32, tag=dst_tag, name=f"h{l}{which}")
        for ct in range(CT):
            nc.vector.tensor_tensor(out=h[:, ct, :], in0=resid[:, ct, :],
                                    in1=psR[:], op=OP.mult)
            nc.vector.tensor_tensor(out=h[:, ct, :], in0=h[:, ct, :],
                                    in1=psM[:], op=OP.add)
            nc.vector.tensor_scalar(out=h[:, ct, :], in0=h[:, ct, :],
                                    scalar1=g[:, ct:ct + 1], scalar2=b[:, ct:ct + 1],
                                    op0=OP.mult, op1=OP.add)
        return h

    def matmul_block(dst, dst_slice_fn, w_dram, h, kt, n_cols, l, nm,
                     act=None, bias=None, chunk_cols=512):
        """dst[.., n] = act(w.T @ h + bias); w_dram [P, kt, n_cols] fp32.
        kt = contraction tiles; streams weight chunks of chunk_cols."""
        for c0 in range(0, n_cols, chunk_cols):
            cw = min(chunk_cols, n_cols - c0)
            wc = wp.tile([P, kt, chunk_cols], F32, tag="w", name=f"w{nm}_{c0}")
            nc.sync.dma_start(out=wc[:, :, :cw], in_=w_dram[:, :, c0:c0 + cw])
            for ni in range(0, cw, P):
                psx = ps_mm.tile([P, T], F32, tag="mm", name=f"ps{nm}_{c0}_{ni}")
                for k in range(kt):
                    nc.tensor.matmul(out=psx[:], lhsT=wc[:, k, ni:ni + P],
                                     rhs=h[:, k, :], start=(k == 0),
                                     stop=(k == kt - 1))
                n_idx = (c0 + ni) // P
                dslice = dst_slice_fn(dst, n_idx)
                if act is not None:
                    bb = bias[:, n_idx:n_idx + 1] if bias is not None else 0.0
                    nc.scalar.activation(out=dslice, in_=psx[:], func=act, bias=bb)
                elif bias is not None:
                    nc.vector.tensor_scalar_add(out=dslice, in0=psx[:],
                                                scalar1=bias[:, n_idx:n_idx + 1])
                else:
                    nc.vector.tensor_copy(out=dslice, in_=psx[:])

    for l in range(L):
        # ---- LN1 + attention ----
        h = layer_norm(l, 1, "h")
        # qkT feature-major: q tiles 0..7 stay; k tiles 8..15 -> bounce
        qT = big.tile([P, CT, T], F32, tag="qT", name=f"qT{l}")
        kTl = big.tile([P, CT, T], F32, tag="big4a", name=f"kTl{l}")
        qkv_d = W[f"l{l}_qkvw"]
        matmul_block(qT, lambda d, n: d[:, n, :], qkv_d[:, :, 0:1024], h, CT,
                     1024, l, f"q{l}")
        matmul_block(kTl, lambda d, n: d[:, n, :], qkv_d[:, :, 1024:2048], h,
                     CT, 1024, l, f"k{l}")
        # v token-major: [tok128 x 4, 1024]
        vloc = big.tile([P, 4, 1024], F32, tag="big4b", name=f"vloc{l}")
        for nv in range(2):
            wc = wp.tile([P, CT, 512], F32, tag="w", name=f"wv{l}_{nv}")
            nc.sync.dma_start(out=wc[:],
                              in_=qkv_d[:, :, 2048 + nv * 512: 2048 + (nv + 1) * 512])
            for m in range(4):
                psv = ps_mm.tile([P, T], F32, tag="mm", name=f"psv{l}_{m}_{nv}")
                for k in range(CT):
                    nc.tensor.matmul(out=psv[:], lhsT=h[:, k, m * P:(m + 1) * P],
                                     rhs=wc[:, k, :], start=(k == 0),
                                     stop=(k == CT - 1))
                nc.vector.tensor_copy(out=vloc[:, m, nv * 512:(nv + 1) * 512],
                                      in_=psv[:])
        # bounce kT + v to DRAM, AllGather, read back my batch pair
        cin = dram.tile([P, 8192], F32, tag="cin", name=f"cin{l}")
        cout = dram.tile([NC_ * P, 8192], F32, tag="cout", name=f"cout{l}")
        nc.sync.dma_start(out=cin[:, 0:4096],
                          in_=kTl[:].rearrange("p c t -> p (c t)"))
        nc.sync.dma_start(out=cin[:, 4096:8192],
                          in_=vloc[:].rearrange("p c t -> p (c t)"))
        nc.gpsimd.collective_compute(
            "AllGather", OP.bypass, replica_groups=[list(range(NC_))],
            ins=[cin.opt()], outs=[cout.opt()])
        kTf = big.tile([P, CT, 1024], F32, tag="big4a", name=f"kTf{l}")
        vaug = big.tile([P, 8, H, 65], F32, tag="big4b", name=f"vaug{l}")
        nc.vector.memset(vaug[:, :, :, 64:65], 1.0)
        for r2 in range(2):
            src = cout[bass.ds(pair_base + r2 * P, P), :]
            nc.sync.dma_start(
                out=kTf[:, :, r2 * 512:(r2 + 1) * 512],
                in_=src[:, 0:4096].rearrange("p (c t) -> p c t", c=CT))
            nc.sync.dma_start(
                out=vaug[:, r2 * 4:(r2 + 1) * 4, :, 0:64],
                in_=src[:, 4096:8192].rearrange("p (c h d) -> p c h d", c=4, h=H))
        # attention per head; heads 2hp/2hp+1 share c-tile hp (rows 0-63/64-127)
        attT = big.tile([P, CT, T], F32, tag="tmp2m", name=f"attT{l}")
        for hd in range(H):
            hp, half = hd // 2, hd % 2
            rows = slice(half * 64, half * 64 + 64)
            psA = ps_av.tile([65, T], F32, tag="av", name=f"psA{l}_{hd}")
            for kc in range(8):
                psS = ps_aux.tile([P, T], F32, tag="aux", name=f"psS{l}_{hd}_{kc}")
                nc.tensor.matmul(out=psS[:], lhsT=kTf[rows, hp, kc * P:(kc + 1) * P],
                                 rhs=qT[rows, hp, :], start=True, stop=True)
                pt = ptp.tile([P, T], F32, tag="pt", name=f"pt{l}_{hd}_{kc}")
                nc.scalar.activation(out=pt[:], in_=psS[:], func=AF.Exp, scale=0.125)
                nc.tensor.matmul(out=psA[:], lhsT=vaug[:, kc, hd, :], rhs=pt[:],
                                 start=(kc == 0), stop=(kc == 7))
            av = sm2.tile([65, T], F32, tag="avs", name=f"av{l}_{hd}")
            nc.vector.tensor_copy(out=av[:], in_=psA[:])
            rec = sm2.tile([1, T], F32, tag="rec", name=f"rec{l}_{hd}")
            nc.vector.reciprocal(out=rec[:], in_=av[64:65, :])
            psB = ps_aux.tile([64, T], F32, tag="aux", name=f"psB{l}_{hd}")
            nc.tensor.matmul(out=psB[:], lhsT=ones_r[:, 0:64], rhs=rec[:],
                             start=True, stop=True)
            nc.vector.tensor_tensor(out=attT[rows, hp, :], in0=av[0:64, :],
                                    in1=psB[:], op=OP.mult)
        # out-projection + residual add
        outb = sm.tile([P, CT], F32, tag="lnb2", name=f"outb{l}")
        nc.sync.dma_start(out=outb[:], in_=W[f"l{l}_outb"][:])
        ow_d = W[f"l{l}_outw"]
        for c0 in (0, 512):
            wc = wp.tile([P, CT, 512], F32, tag="w", name=f"wo{l}_{c0}")
            nc.sync.dma_start(out=wc[:], in_=ow_d[:, :, c0:c0 + 512])
            for ni in range(4):
                ct = (c0 + ni * P) // P
                psx = ps_mm.tile([P, T], F32, tag="mm", name=f"pso{l}_{ct}")
                for k in range(CT):
                    nc.tensor.matmul(out=psx[:], lhsT=wc[:, k, ni * P:(ni + 1) * P],
                                     rhs=attT[:, k, :], start=(k == 0),
                                     stop=(k == CT - 1))
                tb = sm2.tile([P, T], F32, tag="projtmp", name=f"tb{l}_{ct}")
                nc.vector.tensor_scalar_add(out=tb[:], in0=psx[:],
                                            scalar1=outb[:, ct:ct + 1])
                nc.vector.tensor_tensor(out=resid[:, ct, :], in0=resid[:, ct, :],
                                        in1=tb[:], op=OP.add)

        # ---- LN2 + FFN/MoE ----
        h2 = layer_norm(l, 2, "h")
        if l < NDENSE:
            ffb1 = sm.tile([P, HT], F32, tag="lnb2", name=f"ffb1{l}")
            nc.sync.dma_start(out=ffb1[:], in_=W[f"l{l}_ffb1"][:])
            f1 = big.tile([P, HT, T], F32, tag="big4a", name=f"f1{l}")
            matmul_block(f1, lambda d, n: d[:, n, :], W[f"l{l}_ffw1"], h2, CT,
                         HID, l, f"ff1{l}", act=AF.Gelu, bias=ffb1)
            ffb2 = sm.tile([P, CT], F32, tag="lnb2", name=f"ffb2{l}")
            nc.sync.dma_start(out=ffb2[:], in_=W[f"l{l}_ffb2"][:])
            f2w = W[f"l{l}_ffw2"]
            for ct in range(CT):
                psx = ps_mm.tile([P, T], F32, tag="mm", name=f"psf2{l}_{ct}")
                for kh in range(2):
                    wc = wp.tile([P, CT, 512], F32, tag="w", name=f"wf2{l}_{ct}_{kh}")
                    nc.sync.dma_start(
                        out=wc[:, :, 0:P],
                        in_=f2w[:, kh * CT:(kh + 1) * CT, ct * P:(ct + 1) * P])
                    for k in range(CT):
                        nc.tensor.matmul(out=psx[:], lhsT=wc[:, k, 0:P],
                                         rhs=f1[:, kh * CT + k, :],
                                         start=(kh == 0 and k == 0),
                                         stop=(kh == 1 and k == CT - 1))
                tb = sm2.tile([P, T], F32, tag="projtmp", name=f"tf{l}_{ct}")
                nc.vector.tensor_scalar_add(out=tb[:], in0=psx[:],
                                            scalar1=ffb2[:, ct:ct + 1])
                nc.vector.tensor_tensor(out=resid[:, ct, :], in0=resid[:, ct, :],
                                        in1=tb[:], op=OP.add)
        else:
            _moe(nc, tc, l, W, h2, resid, ident, ones_r, big, wp, sm, sm2,
                 ps_mm, ps_aux, ps_st)

    nc.sync.dma_start(out=out_d[:], in_=resid[:])
    ctx.close()


def _moe(nc, tc, l, W, h2, resid, ident, ones_r, big, wp, sm, sm2, ps_mm, ps_aux, ps_st):
    # gate logits [E, T] feature-major
    gw = sm.tile([P, CT, E], F32, tag="gw", name=f"gw{l}")
    nc.sync.dma_start(out=gw[:], in_=W[f"l{l}_gwT"][:])
    psg = ps_st.tile([E, T], F32, tag="st", name=f"psg{l}")
    for k in range(CT):
        nc.tensor.matmul(out=psg[:], lhsT=gw[:, k, :], rhs=h2[:, k, :],
                         start=(k == 0), stop=(k == CT - 1))
    lg = sm.tile([E, T], F32, tag="lg", name=f"lg{l}")
    nc.vector.tensor_copy(out=lg[:], in_=psg[:])
    # transpose to token-major [128, 4, E]
    lgT = sm.tile([P, 4, E], F32, tag="lgT", name=f"lgT{l}")
    for j in range(4):
        pst = ps_st.tile([P, E], F32, tag="st", name=f"pst{l}_{j}")
        nc.tensor.transpose(out=pst[:], in_=lg[:, j * P:(j + 1) * P],
                            identity=ident[0:E, 0:E])
        nc.vector.tensor_copy(out=lgT[:, j, :], in_=pst[:])
    # top-2 mask + softmax (max-subtracted, matching reference)
    wk = sm.tile([P, 4, 6, E], F32, tag="wk", name=f"wk{l}")
    m1 = sm.tile([P, 4, 4], F32, tag="m1", name=f"m1{l}")
    for j in range(4):
        nc.vector.tensor_reduce(out=m1[:, j, 0:1], in_=lgT[:, j, :],
                                axis=mybir.AxisListType.X, op=OP.max)
        # eq-mask of the max, knock it out, then second max
        nc.vector.tensor_scalar(out=wk[:, j, 0, :], in0=lgT[:, j, :],
                                scalar1=m1[:, j, 0:1], scalar2=None,
                                op0=OP.is_equal)
        nc.vector.tensor_scalar_mul(out=wk[:, j, 1, :], in0=wk[:, j, 0, :],
                                    scalar1=1e30)
        nc.vector.tensor_tensor(out=wk[:, j, 1, :], in0=lgT[:, j, :],
                                in1=wk[:, j, 1, :], op=OP.subtract)
        nc.vector.tensor_reduce(out=m1[:, j, 1:2], in_=wk[:, j, 1, :],
                                axis=mybir.AxisListType.X, op=OP.max)
        nc.vector.tensor_scalar(out=wk[:, j, 2, :], in0=lgT[:, j, :],
                                scalar1=m1[:, j, 1:2], scalar2=None,
                                op0=OP.is_ge)
        # softmax exp(x - max)
        nc.vector.tensor_scalar_mul(out=m1[:, j, 2:3], in0=m1[:, j, 0:1],
                                    scalar1=-1.0)
        nc.scalar.activation(out=wk[:, j, 3, :], in_=lgT[:, j, :], func=AF.Exp,
                             bias=m1[:, j, 2:3])
        nc.vector.tensor_reduce(out=m1[:, j, 3:4], in_=wk[:, j, 3, :],
                                axis=mybir.AxisListType.X, op=OP.add)
        nc.vector.reciprocal(out=m1[:, j, 3:4], in_=m1[:, j, 3:4])
        nc.vector.tensor_mul(out=wk[:, j, 4, :], in0=wk[:, j, 3, :],
                             in1=wk[:, j, 2, :])
        nc.vector.tensor_scalar_mul(out=wk[:, j, 5, :], in0=wk[:, j, 4, :],
                                    scalar1=m1[:, j, 3:4])
    # weT [E, T] feature-major combine weights
    weT = sm.tile([E, T], F32, tag="lg2", name=f"weT{l}")
    for j in range(4):
        pst = ps_st.tile([E, P], F32, tag="st", name=f"psu{l}_{j}")
        nc.tensor.transpose(out=pst[:], in_=wk[:, j, 5, :], identity=ident[:])
        nc.vector.tensor_copy(out=weT[:, j * P:(j + 1) * P], in_=pst[:])

    moe_out = big.tile([P, CT, T], F32, tag="qT", name=f"moeout{l}")
    b2mode = (l < L - 1)
    # split h2 into bf16 hi (+ lo for the compensated layers)
    h2p = big.tile([P, 2, CT, T], BF16, tag="tmp2m", name=f"h2p{l}")
    for ct in range(CT):
        nc.vector.tensor_copy(out=h2p[:, 0, ct, :], in_=h2[:, ct, :])
        if b2mode:
            nc.vector.tensor_tensor(out=h2p[:, 1, ct, :], in0=h2[:, ct, :],
                                    in1=h2p[:, 0, ct, :], op=OP.subtract)

    def mm_terms(psx, wh, wl, rh, rl, k, kt):
        n_terms = 3 if b2mode else 1
        first = (k == 0)
        last = (k == kt - 1)
        nc.tensor.matmul(out=psx[:], lhsT=wh, rhs=rh, start=first,
                         stop=(last and n_terms == 1))
        if b2mode:
            nc.tensor.matmul(out=psx[:], lhsT=wh, rhs=rl, start=False, stop=False)
            nc.tensor.matmul(out=psx[:], lhsT=wl, rhs=rh, start=False, stop=last)

    for e in range(E):
        b1 = sm.tile([P, HT], F32, tag="lnb2", name=f"exb1{l}_{e}")
        b3 = sm.tile([P, HT], F32, tag="lnb3", name=f"exb3{l}_{e}")
        nc.sync.dma_start(out=b1[:], in_=W[f"l{l}_exb1"][e])
        nc.sync.dma_start(out=b3[:], in_=W[f"l{l}_exb3"][e])
        g1 = big.tile([P, HT, T], F32, tag="big4a", name=f"g1_{l}_{e}")
        g3 = big.tile([P, HT, T], F32, tag="big4b", name=f"g3_{l}_{e}")
        w1d, w3d, w2d = (W[f"l{l}_exw1"][e], W[f"l{l}_exw3"][e], W[f"l{l}_exw2"][e])
        for c0 in range(0, HID, 512):
            for (wd, gg, bb, acts) in ((w1d, g1, b1, AF.Silu), (w3d, g3, b3, None)):
                if b2mode:
                    wc = wp.tile([P, CT, 2, 512], BF16, tag="w",
                                 name=f"we{l}_{e}_{c0}_{acts}")
                    nc.sync.dma_start(out=wc[:], in_=wd[:, :, :, c0:c0 + 512])
                else:
                    wc = wp.tile([P, CT, 512], BF16, tag="w",
                                 name=f"we{l}_{e}_{c0}_{acts}")
                    nc.sync.dma_start(out=wc[:], in_=wd[:, :, c0:c0 + 512])
                for ni in range(4):
                    nt = (c0 + ni * P) // P
                    psx = ps_mm.tile([P, T], F32, tag="mm",
                                     name=f"pse{l}_{e}_{nt}_{acts}")
                    for k in range(CT):
                        if b2mode:
                            wh, wl = wc[:, k, 0, ni * P:(ni + 1) * P], wc[:, k, 1, ni * P:(ni + 1) * P]
                        else:
                            wh = wl = wc[:, k, ni * P:(ni + 1) * P]
                        mm_terms(psx, wh, wl, h2p[:, 0, k, :], h2p[:, 1, k, :], k, CT)
                    if acts is not None:
                        nc.scalar.activation(out=gg[:, nt, :], in_=psx[:],
                                             func=acts, bias=bb[:, nt:nt + 1])
                    else:
                        nc.vector.tensor_scalar_add(out=gg[:, nt, :], in0=psx[:],
                                                    scalar1=bb[:, nt:nt + 1])
        # we_e broadcast (K=1 matmul; row DMA'd to partition 0 first)
        werow = sm2.tile([1, T], F32, tag="werow", name=f"werow{l}_{e}")
        nc.sync.dma_start(out=werow[:], in_=weT[e:e + 1, :])
        psW = ps_aux.tile([P, T], F32, tag="aux", name=f"psW{l}_{e}")
        nc.tensor.matmul(out=psW[:], lhsT=ones_r[:], rhs=werow[:],
                         start=True, stop=True)
        # ge = silu(g1)*g3*we -> f32 in g3, then bf16 hi/lo for the w2 matmul
        for k in range(HT):
            nc.vector.tensor_tensor(out=g1[:, k, :], in0=g1[:, k, :],
                                    in1=g3[:, k, :], op=OP.mult)
            nc.vector.tensor_tensor(out=g3[:, k, :], in0=g1[:, k, :],
                                    in1=psW[:], op=OP.mult)
        gehl = big.tile([P, 2, HT, T], BF16, tag="big4a", name=f"gehl{l}_{e}")
        for k in range(HT):
            nc.vector.tensor_copy(out=gehl[:, 0, k, :], in_=g3[:, k, :])
            if b2mode:
                nc.vector.tensor_tensor(out=gehl[:, 1, k, :], in0=g3[:, k, :],
                                        in1=gehl[:, 0, k, :], op=OP.subtract)
        # w2: [HID -> C], accumulate over experts in moe_out
        for ct in range(CT):
            psx = ps_mm.tile([P, T], F32, tag="mm", name=f"ps2{l}_{e}_{ct}")
            for kh in range(2):
                if b2mode:
                    wc = wp.tile([P, CT, 2, P], BF16, tag="w",
                                 name=f"w2{l}_{e}_{ct}_{kh}")
                    nc.sync.dma_start(
                        out=wc[:],
                        in_=w2d[:, kh * CT:(kh + 1) * CT, :, ct * P:(ct + 1) * P])
                else:
                    wc = wp.tile([P, CT, P], BF16, tag="w",
                                 name=f"w2{l}_{e}_{ct}_{kh}")
                    nc.sync.dma_start(
                        out=wc[:],
                        in_=w2d[:, kh * CT:(kh + 1) * CT, ct * P:(ct + 1) * P])
                for k in range(CT):
                    if b2mode:
                        wh, wl = wc[:, k, 0, :], wc[:, k, 1, :]
                    else:
                        wh = wl = wc[:, k, :]
                    kk = kh * CT + k
                    mm_terms(psx, wh, wl, gehl[:, 0, kk, :], gehl[:, 1, kk, :],
                             kk, HT)
            if e == 0:
                nc.vector.tensor_copy(out=moe_out[:, ct, :], in_=psx[:])
            else:
                nc.vector.tensor_tensor(out=moe_out[:, ct, :],
                                        in0=moe_out[:, ct, :], in1=psx[:],
                                        op=OP.add)
    # bias contribution: sum_e we_e * b2_e  == exb2r.T @ weT  (contraction E)
    b2r = sm.tile([E, CT, P], F32, tag="b2r", name=f"b2r{l}")
    nc.sync.dma_start(out=b2r[:], in_=W[f"l{l}_exb2r"][:])
    for ct in range(CT):
        psx = ps_aux.tile([P, T], F32, tag="aux", name=f"psb2{l}_{ct}")
        nc.tensor.matmul(out=psx[:], lhsT=b2r[:, ct, :], rhs=weT[:],
                         start=True, stop=True)
        nc.vector.tensor_tensor(out=moe_out[:, ct, :], in0=moe_out[:, ct, :],
                                in1=psx[:], op=OP.add)
        nc.vector.tensor_tensor(out=resid[:, ct, :], in0=resid[:, ct, :],
                                in1=moe_out[:, ct, :], op=OP.add)


def _stage_weights(params):
    fl = {}
    for l, p in enumerate(params):
        g = lambda k: np.ascontiguousarray(np.asarray(p[k], dtype=np.float32))
        fl[f"l{l}_ln1g"] = g("ln1_g").reshape(CT, P).T.copy()
        fl[f"l{l}_ln1b"] = g("ln1_b").reshape(CT, P).T.copy()
        fl[f"l{l}_ln2g"] = g("ln2_g").reshape(CT, P).T.copy()
        fl[f"l{l}_ln2b"] = g("ln2_b").reshape(CT, P).T.copy()
        fl[f"l{l}_qkvw"] = g("qkv_w").reshape(CT, P, 3 * H * D).transpose(1, 0, 2).copy()
        fl[f"l{l}_outw"] = g("out_w").reshape(CT, P, C).transpose(1, 0, 2).copy()
        fl[f"l{l}_outb"] = g("out_b").reshape(CT, P).T.copy()
        if "ff_w1" in p:
            fl[f"l{l}_ffw1"] = g("ff_w1").reshape(CT, P, HID).transpose(1, 0, 2).copy()
            fl[f"l{l}_ffb1"] = g("ff_b1").reshape(HT, P).T.copy()
            fl[f"l{l}_ffw2"] = g("ff_w2").reshape(HT, P, C).transpose(1, 0, 2).copy()
            fl[f"l{l}_ffb2"] = g("ff_b2").reshape(CT, P).T.copy()
        else:
            fl[f"l{l}_gwT"] = np.ascontiguousarray(g("gate_w").T).reshape(CT, P, E).transpose(1, 0, 2).copy()
            import ml_dtypes
            bf = ml_dtypes.bfloat16
            w1 = g("ex_w1").reshape(E, CT, P, HID).transpose(0, 2, 1, 3)
            w3 = g("ex_w3").reshape(E, CT, P, HID).transpose(0, 2, 1, 3)
            w2 = g("ex_w2").reshape(E, HT, P, C).transpose(0, 2, 1, 3)
            if l < L - 1:
                def split(w):
                    wh = w.astype(bf)
                    wl = (w - wh.astype(np.float32)).astype(bf)
                    return np.ascontiguousarray(np.stack([wh, wl], axis=3))
                fl[f"l{l}_exw1"] = split(w1)
                fl[f"l{l}_exw3"] = split(w3)
                fl[f"l{l}_exw2"] = split(w2)
            else:
                fl[f"l{l}_exw1"] = np.ascontiguousarray(w1.astype(bf))
                fl[f"l{l}_exw3"] = np.ascontiguousarray(w3.astype(bf))
                fl[f"l{l}_exw2"] = np.ascontiguousarray(w2.astype(bf))
            fl[f"l{l}_exb1"] = g("ex_b1").reshape(E, HT, P).transpose(0, 2, 1).copy()
            fl[f"l{l}_exb3"] = g("ex_b3").reshape(E, HT, P).transpose(0, 2, 1).copy()
            fl[f"l{l}_exb2r"] = g("ex_b2").reshape(E, CT, P).copy()
    return fl


_NC_CACHE = None


def kernel(x, params):
    global _NC_CACHE, LAST_RESULT
    x = np.asarray(x, dtype=np.float32)
    staged = _stage_weights(params)
    if _NC_CACHE is None:
        _NC_CACHE = _build()
    nc = _NC_CACHE
    xr = x.reshape(B, 2, T, C)
    in_maps = []
    for c in range(NC_):
        shard = xr[c // 2, c % 2]                      # [T, C]
        xt = np.ascontiguousarray(shard.T.reshape(CT, P, T).transpose(1, 0, 2))
        m = {"xt": xt}
        m.update(staged)
        in_maps.append(m)
    res = run_bass_kernel_spmd(nc, in_maps, list(range(NC_)),
                               trace=bool(os.environ.get("BASS_TRACE")))
    LAST_RESULT = res
    out = np.empty((B, 2, T, C), np.float32)
    for c in range(NC_):
        ot = res.results[c]["out_t"]                   # [P, CT, T]
        out[c // 2, c % 2] = ot.transpose(1, 0, 2).reshape(C, T).T
    return out.reshape(B, N, C)


# revision 13
# speedup vs baseline: 1.3998x; 1.0431x over previous
"""Trainium2 Bass kernel: 4-layer MoE transformer decoder (B=4,N=1024,C=1024,
H=16,D=64,HID=2048, layer0 dense GELU FFN, layers1-3 MoE E=8 top-2).

Sharding: tokens (B*N=4096) split 8 ways (512/core, core c = batch c//2 half
c%2). Weights replicated. Attention needs full-batch K/V -> one 8-core
AllGather per layer; readback uses partition-id-based dynamic DMA offsets.

Activations are kept feature-major ([C partitions, tokens free]) so every
matmul uses weights as the stationary operand. All matmuls fp32: the MoE gate
top-2 margins go down to 2.6e-6, so reduced-precision matmuls upstream of any
gate flip token routing vs the fp32 reference and blow the absmax error.
"""
import os, sys, types

sys.path.insert(0, "/opt/trn_rl_repo")
try:  # profiling hook (missing module in this image); harmless if absent
    from trn_agent_boot.trn_boot import _ntff_profile_via_ctypes
    if 'antenv.axon_hooks' not in sys.modules:
        _m = types.ModuleType('antenv.axon_hooks')
        _m.get_axon_ntff_profile_hook = (
            lambda: _ntff_profile_via_ctypes('/opt/axon/libaxon_pjrt.so'))
        sys.modules['antenv.axon_hooks'] = _m
except Exception:
    pass

import numpy as np
import concourse.bass as bass
import concourse.tile as tile
from concourse import bacc, mybir
from concourse.bass_utils import run_bass_kernel_spmd
from concourse.masks import make_identity

F32 = mybir.dt.float32
BF16 = mybir.dt.bfloat16
AF = mybir.ActivationFunctionType
OP = mybir.AluOpType

B, N, C = 4, 1024, 1024
H, D = 16, 64
HID = 2048
L, NDENSE = 4, 1
E, TOPK = 8, 2
NC_ = 8              # cores
T = 512              # tokens per core
P = 128
CT = C // P          # 8 c-tiles
HT = HID // P        # 16 hid-tiles
EPS = 1e-5

LAST_RESULT = None   # test.py reads exec_time_ns from here


def _build():
    nc = bacc.Bacc("TRN2", target_bir_lowering=False, debug=False,
                   num_devices=NC_)
    dp = nc.declare_dram_parameter
    xt_in = dp("xt", [P, CT, T], F32, isOutput=False)
    out_d = dp("out_t", [P, CT, T], F32, isOutput=True)
    W = {}
    for l in range(L):
        W[f"l{l}_ln1g"] = dp(f"l{l}_ln1g", [P, CT], F32, isOutput=False)
        W[f"l{l}_ln1b"] = dp(f"l{l}_ln1b", [P, CT], F32, isOutput=False)
        W[f"l{l}_ln2g"] = dp(f"l{l}_ln2g", [P, CT], F32, isOutput=False)
        W[f"l{l}_ln2b"] = dp(f"l{l}_ln2b", [P, CT], F32, isOutput=False)
        W[f"l{l}_qkvw"] = dp(f"l{l}_qkvw", [P, CT, 2, 3 * H * D], BF16, isOutput=False)
        W[f"l{l}_outw"] = dp(f"l{l}_outw", [P, CT, 2, C], BF16, isOutput=False)
        W[f"l{l}_outb"] = dp(f"l{l}_outb", [P, CT], F32, isOutput=False)
        if l < NDENSE:
            W[f"l{l}_ffw1"] = dp(f"l{l}_ffw1", [P, CT, 2, HID], BF16, isOutput=False)
            W[f"l{l}_ffb1"] = dp(f"l{l}_ffb1", [P, HT], F32, isOutput=False)
            W[f"l{l}_ffw2"] = dp(f"l{l}_ffw2", [P, HT, 2, C], BF16, isOutput=False)
            W[f"l{l}_ffb2"] = dp(f"l{l}_ffb2", [P, CT], F32, isOutput=False)
        else:
            W[f"l{l}_gwT"] = dp(f"l{l}_gwT", [P, CT, E], F32, isOutput=False)
            if l < L - 1:   # hi/lo bf16 pairs (3-term compensated matmuls)
                W[f"l{l}_exw1"] = dp(f"l{l}_exw1", [E, P, CT, 2, HID], BF16, isOutput=False)
                W[f"l{l}_exw3"] = dp(f"l{l}_exw3", [E, P, CT, 2, HID], BF16, isOutput=False)
                W[f"l{l}_exw2"] = dp(f"l{l}_exw2", [E, P, HT, 2, C], BF16, isOutput=False)
            else:           # last layer feeds no gate: plain bf16
                W[f"l{l}_exw1"] = dp(f"l{l}_exw1", [E, P, CT, HID], BF16, isOutput=False)
                W[f"l{l}_exw3"] = dp(f"l{l}_exw3", [E, P, CT, HID], BF16, isOutput=False)
                W[f"l{l}_exw2"] = dp(f"l{l}_exw2", [E, P, HT, C], BF16, isOutput=False)
            W[f"l{l}_exb1"] = dp(f"l{l}_exb1", [E, P, HT], F32, isOutput=False)
            W[f"l{l}_exb3"] = dp(f"l{l}_exb3", [E, P, HT], F32, isOutput=False)
            W[f"l{l}_exb2r"] = dp(f"l{l}_exb2r", [E, CT, P], F32, isOutput=False)

    with tile.TileContext(nc) as tc:
        _emit(nc, tc, xt_in, out_d, W)
    nc.compile()
    return nc


def _emit(nc, tc, xt_in, out_d, W):
    from contextlib import ExitStack
    ctx = ExitStack()
    const = ctx.enter_context(tc.tile_pool(name="const", bufs=1))
    big = ctx.enter_context(tc.tile_pool(name="big", bufs=1))
    wp = ctx.enter_context(tc.tile_pool(name="wp", bufs=2))
    ptp = ctx.enter_context(tc.tile_pool(name="ptp", bufs=3))
    sm = ctx.enter_context(tc.tile_pool(name="sm", bufs=1))
    sm2 = ctx.enter_context(tc.tile_pool(name="sm2", bufs=2))
    dram = ctx.enter_context(tc.tile_pool(name="dram", bufs=1, space="DRAM"))
    ps_mm = ctx.enter_context(tc.tile_pool(name="ps_mm", bufs=2, space="PSUM"))
    ps_aux = ctx.enter_context(tc.tile_pool(name="ps_aux", bufs=2, space="PSUM"))
    ps_av = ctx.enter_context(tc.tile_pool(name="ps_av", bufs=2, space="PSUM"))
    ps_st = ctx.enter_context(tc.tile_pool(name="ps_st", bufs=2, space="PSUM"))

    ident = const.tile([P, P], F32)
    make_identity(nc, ident[:])
    ones_c = const.tile([P, 1], F32)      # column of ones (lhsT for col-sums)
    nc.vector.memset(ones_c[:], 1.0)
    ones_r = const.tile([1, P], F32)      # row of ones (lhsT for broadcasts)
    nc.vector.memset(ones_r[:], 1.0)

    # residual, feature-major [P, CT, T]
    resid = const.tile([P, CT, T], F32)
    nc.sync.dma_start(out=resid[:], in_=xt_in[:])

    pid = nc.sync.partition_id()
    pair_base = (pid // 2) * 2 * P        # AG-row offset of my batch pair

    def layer_norm(l, which, dst_tag):
        """LN over the feature (partition x ctile) axis; returns h tile."""
        g = sm.tile([P, CT], F32, tag="lng", name=f"g{l}{which}")
        b = sm.tile([P, CT], F32, tag="lnb", name=f"b{l}{which}")
        nc.sync.dma_start(out=g[:], in_=W[f"l{l}_ln{which}g"][:])
        nc.sync.dma_start(out=b[:], in_=W[f"l{l}_ln{which}b"][:])
        sq = big.tile([P, CT, T], F32, tag="tmp2m", name=f"sq{l}{which}")
        for ct in range(CT):
            nc.vector.tensor_mul(out=sq[:, ct, :], in0=resid[:, ct, :],
                                 in1=resid[:, ct, :])
        ps1 = ps_st.tile([1, T], F32, tag="st", name=f"ps1_{l}{which}")
        ps2 = ps_st.tile([1, T], F32, tag="st", name=f"ps2_{l}{which}")
        for ct in range(CT):
            nc.tensor.matmul(out=ps1[:], lhsT=ones_c[:], rhs=resid[:, ct, :],
                             start=(ct == 0), stop=(ct == CT - 1))
        for ct in range(CT):
            nc.tensor.matmul(out=ps2[:], lhsT=ones_c[:], rhs=sq[:, ct, :],
                             start=(ct == 0), stop=(ct == CT - 1))
        st = sm.tile([1, 6, T], F32, tag="stats", name=f"st{l}{which}")
        mu, msq, varp, rinv, rstd, numu = (st[:, i, :] for i in range(6))
        nc.vector.tensor_scalar_mul(out=mu, in0=ps1[:], scalar1=1.0 / C)
        nc.vector.tensor_scalar_mul(out=msq, in0=ps2[:], scalar1=1.0 / C)
        nc.vector.tensor_mul(out=varp, in0=mu, in1=mu)
        nc.vector.tensor_tensor(out=varp, in0=msq, in1=varp, op=OP.subtract)
        nc.vector.tensor_scalar_add(out=varp, in0=varp, scalar1=EPS)
        sd = st[:, 1, :]  # reuse msq slot
        nc.scalar.activation(out=sd, in_=varp, func=AF.Sqrt)
        nc.vector.reciprocal(out=rinv, in_=sd)
        # one Newton step: r = rinv*(1.5 - 0.5*varp*rinv^2)
        nc.vector.tensor_mul(out=sd, in0=rinv, in1=rinv)
        nc.vector.tensor_mul(out=sd, in0=sd, in1=varp)
        nc.vector.tensor_scalar(out=sd, in0=sd, scalar1=-0.5, scalar2=1.5,
                                op0=OP.mult, op1=OP.add)
        nc.vector.tensor_mul(out=rstd, in0=rinv, in1=sd)
        nc.vector.tensor_mul(out=numu, in0=mu, in1=rstd)
        nc.vector.tensor_scalar_mul(out=numu, in0=numu, scalar1=-1.0)
        psR = ps_aux.tile([P, T], F32, tag="aux", name=f"psR{l}{which}")
        psM = ps_aux.tile([P, T], F32, tag="aux", name=f"psM{l}{which}")
        nc.tensor.matmul(out=psR[:], lhsT=ones_r[:], rhs=rstd, start=True, stop=True)
        nc.tensor.matmul(out=psM[:], lhsT=ones_r[:], rhs=numu, start=True, stop=True)
        h = big.tile([P, CT, T], F32, tag=dst_tag, name=f"h{l}{which}")
        for ct in range(CT):
            nc.vector.tensor_tensor(out=h[:, ct, :], in0=resid[:, ct, :],
                                    in1=psR[:], op=OP.mult)
            nc.vector.tensor_tensor(out=h[:, ct, :], in0=h[:, ct, :],
                                    in1=psM[:], op=OP.add)
            nc.vector.tensor_scalar(out=h[:, ct, :], in0=h[:, ct, :],
                                    scalar1=g[:, ct:ct + 1], scalar2=b[:, ct:ct + 1],
                                    op0=OP.mult, op1=OP.add)
        return h

    def split_pair(srcf32, kt, tag, nm):
        """f32 [P, kt, T] -> bf16 hi/lo pair [P, 2, kt, T]"""
        hp = big.tile([P, 2, kt, T], BF16, tag=tag, name=f"sp{nm}")
        for k in range(kt):
            nc.vector.tensor_copy(out=hp[:, 0, k, :], in_=srcf32[:, k, :])
            nc.vector.tensor_tensor(out=hp[:, 1, k, :], in0=srcf32[:, k, :],
                                    in1=hp[:, 0, k, :], op=OP.subtract)
        return hp

    def matmul_block(dst, dst_slice_fn, w_dram, hp, kt, n_cols, l, nm,
                     act=None, bias=None, chunk_cols=512):
        """dst[.., n] = act(w.T @ h + bias); w_dram [P, kt, 2, n_cols] bf16
        hi/lo pairs, hp [P, 2, kt, T] bf16 hi/lo; 3-term compensated."""
        for c0 in range(0, n_cols, chunk_cols):
            cw = min(chunk_cols, n_cols - c0)
            wc = wp.tile([P, kt, 2, chunk_cols], BF16, tag="w", name=f"w{nm}_{c0}")
            nc.sync.dma_start(out=wc[:, :, :, :cw], in_=w_dram[:, :, :, c0:c0 + cw])
            for ni in range(0, cw, P):
                psx = ps_mm.tile([P, T], F32, tag="mm", name=f"ps{nm}_{c0}_{ni}")
                for k in range(kt):
                    wh, wl = wc[:, k, 0, ni:ni + P], wc[:, k, 1, ni:ni + P]
                    nc.tensor.matmul(out=psx[:], lhsT=wh, rhs=hp[:, 0, k, :],
                                     start=(k == 0), stop=False)
                    nc.tensor.matmul(out=psx[:], lhsT=wh, rhs=hp[:, 1, k, :],
                                     start=False, stop=False)
                    nc.tensor.matmul(out=psx[:], lhsT=wl, rhs=hp[:, 0, k, :],
                                     start=False, stop=(k == kt - 1))
                n_idx = (c0 + ni) // P
                dslice = dst_slice_fn(dst, n_idx)
                if act is not None:
                    bb = bias[:, n_idx:n_idx + 1] if bias is not None else 0.0
                    nc.scalar.activation(out=dslice, in_=psx[:], func=act, bias=bb)
                elif bias is not None:
                    nc.vector.tensor_scalar_add(out=dslice, in0=psx[:],
                                                scalar1=bias[:, n_idx:n_idx + 1])
                else:
                    nc.vector.tensor_copy(out=dslice, in_=psx[:])

    for l in range(L):
        # ---- LN1 + attention ----
        h = layer_norm(l, 1, "h")
        h1p = split_pair(h, CT, "tmp2m", f"h1p{l}")
        # qkT feature-major: q tiles 0..7 stay; k tiles 8..15 -> bounce
        qT = big.tile([P, CT, T], F32, tag="qT", name=f"qT{l}")
        kTl = big.tile([P, CT, T], F32, tag="big4a", name=f"kTl{l}")
        qkv_d = W[f"l{l}_qkvw"]
        matmul_block(qT, lambda d, n: d[:, n, :], qkv_d[:, :, :, 0:1024], h1p,
                     CT, 1024, l, f"q{l}")
        matmul_block(kTl, lambda d, n: d[:, n, :], qkv_d[:, :, :, 1024:2048],
                     h1p, CT, 1024, l, f"k{l}")
        # v token-major: [tok128 x 4, 1024]; lhsT = h token-chunks (hi/lo)
        vloc = big.tile([P, 4, 1024], F32, tag="big4b", name=f"vloc{l}")
        for nv in range(2):
            wc = wp.tile([P, CT, 2, 512], BF16, tag="w", name=f"wv{l}_{nv}")
            nc.sync.dma_start(out=wc[:],
                              in_=qkv_d[:, :, :, 2048 + nv * 512: 2048 + (nv + 1) * 512])
            for m in range(4):
                psv = ps_mm.tile([P, T], F32, tag="mm", name=f"psv{l}_{m}_{nv}")
                for k in range(CT):
                    lh = h1p[:, 0, k, m * P:(m + 1) * P]
                    ll = h1p[:, 1, k, m * P:(m + 1) * P]
                    nc.tensor.matmul(out=psv[:], lhsT=lh, rhs=wc[:, k, 0, :],
                                     start=(k == 0), stop=False)
                    nc.tensor.matmul(out=psv[:], lhsT=ll, rhs=wc[:, k, 0, :],
                                     start=False, stop=False)
                    nc.tensor.matmul(out=psv[:], lhsT=lh, rhs=wc[:, k, 1, :],
                                     start=False, stop=(k == CT - 1))
                nc.vector.tensor_copy(out=vloc[:, m, nv * 512:(nv + 1) * 512],
                                      in_=psv[:])
        # bounce kT + v to DRAM, AllGather, read back my batch pair
        cin = dram.tile([P, 8192], F32, tag="cin", name=f"cin{l}")
        cout = dram.tile([NC_ * P, 8192], F32, tag="cout", name=f"cout{l}")
        nc.sync.dma_start(out=cin[:, 0:4096],
                          in_=kTl[:].rearrange("p c t -> p (c t)"))
        nc.sync.dma_start(out=cin[:, 4096:8192],
                          in_=vloc[:].rearrange("p c t -> p (c t)"))
        nc.gpsimd.collective_compute(
            "AllGather", OP.bypass, replica_groups=[list(range(NC_))],
            ins=[cin.opt()], outs=[cout.opt()])
        kTf = big.tile([P, CT, 1024], F32, tag="big4a", name=f"kTf{l}")
        vaug = big.tile([P, 8, H, 65], F32, tag="big4b", name=f"vaug{l}")
        nc.vector.memset(vaug[:, :, :, 64:65], 1.0)
        for r2 in range(2):
            src = cout[bass.ds(pair_base + r2 * P, P), :]
            nc.sync.dma_start(
                out=kTf[:, :, r2 * 512:(r2 + 1) * 512],
                in_=src[:, 0:4096].rearrange("p (c t) -> p c t", c=CT))
            nc.sync.dma_start(
                out=vaug[:, r2 * 4:(r2 + 1) * 4, :, 0:64],
                in_=src[:, 4096:8192].rearrange("p (c h d) -> p c h d", c=4, h=H))
        # attention per head; heads 2hp/2hp+1 share c-tile hp (rows 0-63/64-127)
        attT = big.tile([P, CT, T], F32, tag="tmp2m", name=f"attT{l}")
        for hd in range(H):
            hp, half = hd // 2, hd % 2
            rows = slice(half * 64, half * 64 + 64)
            psA = ps_av.tile([65, T], F32, tag="av", name=f"psA{l}_{hd}")
            for kc in range(8):
                psS = ps_aux.tile([P, T], F32, tag="aux", name=f"psS{l}_{hd}_{kc}")
                nc.tensor.matmul(out=psS[:], lhsT=kTf[rows, hp, kc * P:(kc + 1) * P],
                                 rhs=qT[rows, hp, :], start=True, stop=True)
                pt = ptp.tile([P, T], F32, tag="pt", name=f"pt{l}_{hd}_{kc}")
                nc.scalar.activation(out=pt[:], in_=psS[:], func=AF.Exp, scale=0.125)
                nc.tensor.matmul(out=psA[:], lhsT=vaug[:, kc, hd, :], rhs=pt[:],
                                 start=(kc == 0), stop=(kc == 7))
            av = sm2.tile([65, T], F32, tag="avs", name=f"av{l}_{hd}")
            nc.vector.tensor_copy(out=av[:], in_=psA[:])
            rec = sm2.tile([1, T], F32, tag="rec", name=f"rec{l}_{hd}")
            nc.vector.reciprocal(out=rec[:], in_=av[64:65, :])
            psB = ps_aux.tile([64, T], F32, tag="aux", name=f"psB{l}_{hd}")
            nc.tensor.matmul(out=psB[:], lhsT=ones_r[:, 0:64], rhs=rec[:],
                             start=True, stop=True)
            nc.vector.tensor_tensor(out=attT[rows, hp, :], in0=av[0:64, :],
                                    in1=psB[:], op=OP.mult)
        # out-projection + residual add
        outb = sm.tile([P, CT], F32, tag="lnb2", name=f"outb{l}")
        nc.sync.dma_start(out=outb[:], in_=W[f"l{l}_outb"][:])
        attTp = split_pair(attT, CT, "h", f"attTp{l}")
        ow_d = W[f"l{l}_outw"]
        for c0 in (0, 512):
            wc = wp.tile([P, CT, 2, 512], BF16, tag="w", name=f"wo{l}_{c0}")
            nc.sync.dma_start(out=wc[:], in_=ow_d[:, :, :, c0:c0 + 512])
            for ni in range(4):
                ct = (c0 + ni * P) // P
                psx = ps_mm.tile([P, T], F32, tag="mm", name=f"pso{l}_{ct}")
                for k in range(CT):
                    wh, wl = wc[:, k, 0, ni * P:(ni + 1) * P], wc[:, k, 1, ni * P:(ni + 1) * P]
                    nc.tensor.matmul(out=psx[:], lhsT=wh, rhs=attTp[:, 0, k, :],
                                     start=(k == 0), stop=False)
                    nc.tensor.matmul(out=psx[:], lhsT=wh, rhs=attTp[:, 1, k, :],
                                     start=False, stop=False)
                    nc.tensor.matmul(out=psx[:], lhsT=wl, rhs=attTp[:, 0, k, :],
                                     start=False, stop=(k == CT - 1))
                tb = sm2.tile([P, T], F32, tag="projtmp", name=f"tb{l}_{ct}")
                nc.vector.tensor_scalar_add(out=tb[:], in0=psx[:],
                                            scalar1=outb[:, ct:ct + 1])
                nc.vector.tensor_tensor(out=resid[:, ct, :], in0=resid[:, ct, :],
                                        in1=tb[:], op=OP.add)

        # ---- LN2 + FFN/MoE ----
        h2 = layer_norm(l, 2, "h")
        if l < NDENSE:
            ffb1 = sm.tile([P, HT], F32, tag="lnb2", name=f"ffb1{l}")
            nc.sync.dma_start(out=ffb1[:], in_=W[f"l{l}_ffb1"][:])
            h2p = split_pair(h2, CT, "tmp2m", f"h2pd{l}")
            f1 = big.tile([P, HT, T], F32, tag="big4a", name=f"f1{l}")
            matmul_block(f1, lambda d, n: d[:, n, :], W[f"l{l}_ffw1"], h2p, CT,
                         HID, l, f"ff1{l}", act=AF.Gelu, bias=ffb1)
            ffb2 = sm.tile([P, CT], F32, tag="lnb2", name=f"ffb2{l}")
            nc.sync.dma_start(out=ffb2[:], in_=W[f"l{l}_ffb2"][:])
            f1p = split_pair(f1, HT, "big4b", f"f1p{l}")
            f2w = W[f"l{l}_ffw2"]
            for ct in range(CT):
                psx = ps_mm.tile([P, T], F32, tag="mm", name=f"psf2{l}_{ct}")
                for kh in range(2):
                    wc = wp.tile([P, CT, 2, P], BF16, tag="w", name=f"wf2{l}_{ct}_{kh}")
                    nc.sync.dma_start(
                        out=wc[:],
                        in_=f2w[:, kh * CT:(kh + 1) * CT, :, ct * P:(ct + 1) * P])
                    for k in range(CT):
                        kk = kh * CT + k
                        wh, wl = wc[:, k, 0, :], wc[:, k, 1, :]
                        nc.tensor.matmul(out=psx[:], lhsT=wh, rhs=f1p[:, 0, kk, :],
                                         start=(kk == 0), stop=False)
                        nc.tensor.matmul(out=psx[:], lhsT=wh, rhs=f1p[:, 1, kk, :],
                                         start=False, stop=False)
                        nc.tensor.matmul(out=psx[:], lhsT=wl, rhs=f1p[:, 0, kk, :],
                                         start=False, stop=(kk == HT - 1))
                tb = sm2.tile([P, T], F32, tag="projtmp", name=f"tf{l}_{ct}")
                nc.vector.tensor_scalar_add(out=tb[:], in0=psx[:],
                                            scalar1=ffb2[:, ct:ct + 1])
                nc.vector.tensor_tensor(out=resid[:, ct, :], in0=resid[:, ct, :],
                                        in1=tb[:], op=OP.add)
        else:
            _moe(nc, tc, l, W, h2, resid, ident, ones_r, big, wp, sm, sm2,
                 ps_mm, ps_aux, ps_st)

    nc.sync.dma_start(out=out_d[:], in_=resid[:])
    ctx.close()


def _moe(nc, tc, l, W, h2, resid, ident, ones_r, big, wp, sm, sm2, ps_mm, ps_aux, ps_st):
    # gate logits [E, T] feature-major
    gw = sm.tile([P, CT, E], F32, tag="gw", name=f"gw{l}")
    nc.sync.dma_start(out=gw[:], in_=W[f"l{l}_gwT"][:])
    psg = ps_st.tile([E, T], F32, tag="st", name=f"psg{l}")
    for k in range(CT):
        nc.tensor.matmul(out=psg[:], lhsT=gw[:, k, :], rhs=h2[:, k, :],
                         start=(k == 0), stop=(k == CT - 1))
    lg = sm.tile([E, T], F32, tag="lg", name=f"lg{l}")
    nc.vector.tensor_copy(out=lg[:], in_=psg[:])
    # transpose to token-major [128, 4, E]
    lgT = sm.tile([P, 4, E], F32, tag="lgT", name=f"lgT{l}")
    for j in range(4):
        pst = ps_st.tile([P, E], F32, tag="st", name=f"pst{l}_{j}")
        nc.tensor.transpose(out=pst[:], in_=lg[:, j * P:(j + 1) * P],
                            identity=ident[0:E, 0:E])
        nc.vector.tensor_copy(out=lgT[:, j, :], in_=pst[:])
    # top-2 mask + softmax (max-subtracted, matching reference)
    wk = sm.tile([P, 4, 6, E], F32, tag="wk", name=f"wk{l}")
    m1 = sm.tile([P, 4, 4], F32, tag="m1", name=f"m1{l}")
    for j in range(4):
        nc.vector.tensor_reduce(out=m1[:, j, 0:1], in_=lgT[:, j, :],
                                axis=mybir.AxisListType.X, op=OP.max)
        # eq-mask of the max, knock it out, then second max
        nc.vector.tensor_scalar(out=wk[:, j, 0, :], in0=lgT[:, j, :],
                                scalar1=m1[:, j, 0:1], scalar2=None,
                                op0=OP.is_equal)
        nc.vector.tensor_scalar_mul(out=wk[:, j, 1, :], in0=wk[:, j, 0, :],
                                    scalar1=1e30)
        nc.vector.tensor_tensor(out=wk[:, j, 1, :], in0=lgT[:, j, :],
                                in1=wk[:, j, 1, :], op=OP.subtract)
        nc.vector.tensor_reduce(out=m1[:, j, 1:2], in_=wk[:, j, 1, :],
                                axis=mybir.AxisListType.X, op=OP.max)
        nc.vector.tensor_scalar(out=wk[:, j, 2, :], in0=lgT[:, j, :],
                                scalar1=m1[:, j, 1:2], scalar2=None,
                                op0=OP.is_ge)
        # softmax exp(x - max)
        nc.vector.tensor_scalar_mul(out=m1[:, j, 2:3], in0=m1[:, j, 0:1],
                                    scalar1=-1.0)
        nc.scalar.activation(out=wk[:, j, 3, :], in_=lgT[:, j, :], func=AF.Exp,
                             bias=m1[:, j, 2:3])
        nc.vector.tensor_reduce(out=m1[:, j, 3:4], in_=wk[:, j, 3, :],
                                axis=mybir.AxisListType.X, op=OP.add)
        nc.vector.reciprocal(out=m1[:, j, 3:4], in_=m1[:, j, 3:4])
        nc.vector.tensor_mul(out=wk[:, j, 4, :], in0=wk[:, j, 3, :],
                             in1=wk[:, j, 2, :])
        nc.vector.tensor_scalar_mul(out=wk[:, j, 5, :], in0=wk[:, j, 4, :],
                                    scalar1=m1[:, j, 3:4])
    # weT [E, T] feature-major combine weights
    weT = sm.tile([E, T], F32, tag="lg2", name=f"weT{l}")
    for j in range(4):
        pst = ps_st.tile([E, P], F32, tag="st", name=f"psu{l}_{j}")
        nc.tensor.transpose(out=pst[:], in_=wk[:, j, 5, :], identity=ident[:])
        nc.vector.tensor_copy(out=weT[:, j * P:(j + 1) * P], in_=pst[:])

    moe_out = big.tile([P, CT, T], F32, tag="qT", name=f"moeout{l}")
    b2mode = (l < L - 1)
    # split h2 into bf16 hi (+ lo for the compensated layers)
    h2p = big.tile([P, 2, CT, T], BF16, tag="tmp2m", name=f"h2p{l}")
    for ct in range(CT):
        nc.vector.tensor_copy(out=h2p[:, 0, ct, :], in_=h2[:, ct, :])
        if b2mode:
            nc.vector.tensor_tensor(out=h2p[:, 1, ct, :], in0=h2[:, ct, :],
                                    in1=h2p[:, 0, ct, :], op=OP.subtract)

    def mm_terms(psx, wh, wl, rh, rl, k, kt):
        n_terms = 3 if b2mode else 1
        first = (k == 0)
        last = (k == kt - 1)
        nc.tensor.matmul(out=psx[:], lhsT=wh, rhs=rh, start=first,
                         stop=(last and n_terms == 1))
        if b2mode:
            nc.tensor.matmul(out=psx[:], lhsT=wh, rhs=rl, start=False, stop=False)
            nc.tensor.matmul(out=psx[:], lhsT=wl, rhs=rh, start=False, stop=last)

    for e in range(E):
        b1 = sm.tile([P, HT], F32, tag="lnb2", name=f"exb1{l}_{e}")
        b3 = sm.tile([P, HT], F32, tag="lnb3", name=f"exb3{l}_{e}")
        nc.sync.dma_start(out=b1[:], in_=W[f"l{l}_exb1"][e])
        nc.sync.dma_start(out=b3[:], in_=W[f"l{l}_exb3"][e])
        g1 = big.tile([P, HT, T], F32, tag="big4a", name=f"g1_{l}_{e}")
        g3 = big.tile([P, HT, T], F32, tag="big4b", name=f"g3_{l}_{e}")
        w1d, w3d, w2d = (W[f"l{l}_exw1"][e], W[f"l{l}_exw3"][e], W[f"l{l}_exw2"][e])
        for c0 in range(0, HID, 512):
            for (wd, gg, bb, acts) in ((w1d, g1, b1, AF.Silu), (w3d, g3, b3, None)):
                if b2mode:
                    wc = wp.tile([P, CT, 2, 512], BF16, tag="w",
                                 name=f"we{l}_{e}_{c0}_{acts}")
                    nc.sync.dma_start(out=wc[:], in_=wd[:, :, :, c0:c0 + 512])
                else:
                    wc = wp.tile([P, CT, 512], BF16, tag="w",
                                 name=f"we{l}_{e}_{c0}_{acts}")
                    nc.sync.dma_start(out=wc[:], in_=wd[:, :, c0:c0 + 512])
                for ni in range(4):
                    nt = (c0 + ni * P) // P
                    psx = ps_mm.tile([P, T], F32, tag="mm",
                                     name=f"pse{l}_{e}_{nt}_{acts}")
                    for k in range(CT):
                        if b2mode:
                            wh, wl = wc[:, k, 0, ni * P:(ni + 1) * P], wc[:, k, 1, ni * P:(ni + 1) * P]
                        else:
                            wh = wl = wc[:, k, ni * P:(ni + 1) * P]
                        mm_terms(psx, wh, wl, h2p[:, 0, k, :], h2p[:, 1, k, :], k, CT)
                    if acts is not None:
                        nc.scalar.activation(out=gg[:, nt, :], in_=psx[:],
                                             func=acts, bias=bb[:, nt:nt + 1])
                    else:
                        nc.vector.tensor_scalar_add(out=gg[:, nt, :], in0=psx[:],
                                                    scalar1=bb[:, nt:nt + 1])
        # we_e broadcast (K=1 matmul; row DMA'd to partition 0 first)
        werow = sm2.tile([1, T], F32, tag="werow", name=f"werow{l}_{e}")
        nc.sync.dma_start(out=werow[:], in_=weT[e:e + 1, :])
        psW = ps_aux.tile([P, T], F32, tag="aux", name=f"psW{l}_{e}")
        nc.tensor.matmul(out=psW[:], lhsT=ones_r[:], rhs=werow[:],
                         start=True, stop=True)
        # ge = silu(g1)*g3*we -> f32 in g3, then bf16 hi/lo for the w2 matmul
        for k in range(HT):
            nc.vector.tensor_tensor(out=g1[:, k, :], in0=g1[:, k, :],
                                    in1=g3[:, k, :], op=OP.mult)
            nc.vector.tensor_tensor(out=g3[:, k, :], in0=g1[:, k, :],
                                    in1=psW[:], op=OP.mult)
        gehl = big.tile([P, 2, HT, T], BF16, tag="big4a", name=f"gehl{l}_{e}")
        for k in range(HT):
            nc.vector.tensor_copy(out=gehl[:, 0, k, :], in_=g3[:, k, :])
            if b2mode:
                nc.vector.tensor_tensor(out=gehl[:, 1, k, :], in0=g3[:, k, :],
                                        in1=gehl[:, 0, k, :], op=OP.subtract)
        # w2: [HID -> C], accumulate over experts in moe_out
        for ct in range(CT):
            psx = ps_mm.tile([P, T], F32, tag="mm", name=f"ps2{l}_{e}_{ct}")
            for kh in range(2):
                if b2mode:
                    wc = wp.tile([P, CT, 2, P], BF16, tag="w",
                                 name=f"w2{l}_{e}_{ct}_{kh}")
                    nc.sync.dma_start(
                        out=wc[:],
                        in_=w2d[:, kh * CT:(kh + 1) * CT, :, ct * P:(ct + 1) * P])
                else:
                    wc = wp.tile([P, CT, P], BF16, tag="w",
                                 name=f"w2{l}_{e}_{ct}_{kh}")
                    nc.sync.dma_start(
                        out=wc[:],
                        in_=w2d[:, kh * CT:(kh + 1) * CT, ct * P:(ct + 1) * P])
                for k in range(CT):
                    if b2mode:
                        wh, wl = wc[:, k, 0, :], wc[:, k, 1, :]
                    else:
                        wh = wl = wc[:, k, :]
                    kk = kh * CT + k
                    mm_terms(psx, wh, wl, gehl[:, 0, kk, :], gehl[:, 1, kk, :],
                             kk, HT)
            if e == 0:
                nc.vector.tensor_copy(out=moe_out[:, ct, :], in_=psx[:])
            else:
                nc.vector.tensor_tensor(out=moe_out[:, ct, :],
                                        in0=moe_out[:, ct, :], in1=psx[:],
                                        op=OP.add)
    # bias contribution: sum_e we_e * b2_e  == exb2r.T @ weT  (contraction E)
    b2r = sm.tile([E, CT, P], F32, tag="b2r", name=f"b2r{l}")
    nc.sync.dma_start(out=b2r[:], in_=W[f"l{l}_exb2r"][:])
    for ct in range(CT):
        psx = ps_aux.tile([P, T], F32, tag="aux", name=f"psb2{l}_{ct}")
        nc.tensor.matmul(out=psx[:], lhsT=b2r[:, ct, :], rhs=weT[:],
                         start=True, stop=True)
        nc.vector.tensor_tensor(out=moe_out[:, ct, :], in0=moe_out[:, ct, :],
                                in1=psx[:], op=OP.add)
        nc.vector.tensor_tensor(out=resid[:, ct, :], in0=resid[:, ct, :],
                                in1=moe_out[:, ct, :], op=OP.add)


def _stage_weights(params):
    fl = {}
    for l, p in enumerate(params):
        g = lambda k: np.ascontiguousarray(np.asarray(p[k], dtype=np.float32))
        fl[f"l{l}_ln1g"] = g("ln1_g").reshape(CT, P).T.copy()
        fl[f"l{l}_ln1b"] = g("ln1_b").reshape(CT, P).T.copy()
        fl[f"l{l}_ln2g"] = g("ln2_g").reshape(CT, P).T.copy()
        fl[f"l{l}_ln2b"] = g("ln2_b").reshape(CT, P).T.copy()
        import ml_dtypes
        _bf = ml_dtypes.bfloat16
        def split2(w):  # [P, kt, n] f32 -> [P, kt, 2, n] bf16 hi/lo
            wh = w.astype(_bf)
            wl = (w - wh.astype(np.float32)).astype(_bf)
            return np.ascontiguousarray(np.stack([wh, wl], axis=2))
        fl[f"l{l}_qkvw"] = split2(g("qkv_w").reshape(CT, P, 3 * H * D).transpose(1, 0, 2))
        fl[f"l{l}_outw"] = split2(g("out_w").reshape(CT, P, C).transpose(1, 0, 2))
        fl[f"l{l}_outb"] = g("out_b").reshape(CT, P).T.copy()
        if "ff_w1" in p:
            fl[f"l{l}_ffw1"] = split2(g("ff_w1").reshape(CT, P, HID).transpose(1, 0, 2))
            fl[f"l{l}_ffb1"] = g("ff_b1").reshape(HT, P).T.copy()
            fl[f"l{l}_ffw2"] = split2(g("ff_w2").reshape(HT, P, C).transpose(1, 0, 2))
            fl[f"l{l}_ffb2"] = g("ff_b2").reshape(CT, P).T.copy()
        else:
            fl[f"l{l}_gwT"] = np.ascontiguousarray(g("gate_w").T).reshape(CT, P, E).transpose(1, 0, 2).copy()
            import ml_dtypes
            bf = ml_dtypes.bfloat16
            w1 = g("ex_w1").reshape(E, CT, P, HID).transpose(0, 2, 1, 3)
            w3 = g("ex_w3").reshape(E, CT, P, HID).transpose(0, 2, 1, 3)
            w2 = g("ex_w2").reshape(E, HT, P, C).transpose(0, 2, 1, 3)
            if l < L - 1:
                def split(w):
                    wh = w.astype(bf)
                    wl = (w - wh.astype(np.float32)).astype(bf)
                    return np.ascontiguousarray(np.stack([wh, wl], axis=3))
                fl[f"l{l}_exw1"] = split(w1)
                fl[f"l{l}_exw3"] = split(w3)
                fl[f"l{l}_exw2"] = split(w2)
            else:
                fl[f"l{l}_exw1"] = np.ascontiguousarray(w1.astype(bf))
                fl[f"l{l}_exw3"] = np.ascontiguousarray(w3.astype(bf))
                fl[f"l{l}_exw2"] = np.ascontiguousarray(w2.astype(bf))
            fl[f"l{l}_exb1"] = g("ex_b1").reshape(E, HT, P).transpose(0, 2, 1).copy()
            fl[f"l{l}_exb3"] = g("ex_b3").reshape(E, HT, P).transpose(0, 2, 1).copy()
            fl[f"l{l}_exb2r"] = g("ex_b2").reshape(E, CT, P).copy()
    return fl


_NC_CACHE = None


def kernel(x, params):
    global _NC_CACHE, LAST_RESULT
    x = np.asarray(x, dtype=np.float32)
    staged = _stage_weights(params)
    if _NC_CACHE is None:
        _NC_CACHE = _build()
    nc = _NC_CACHE
    xr = x.reshape(B, 2, T, C)
    in_maps = []
    for c in range(NC_):
        shard = xr[c // 2, c % 2]                      # [T, C]
        xt = np.ascontiguousarray(shard.T.reshape(CT, P, T).transpose(1, 0, 2))
        m = {"xt": xt}
        m.update(staged)
        in_maps.append(m)
    res = run_bass_kernel_spmd(nc, in_maps, list(range(NC_)),
                               trace=bool(os.environ.get("BASS_TRACE")))
    LAST_RESULT = res
    out = np.empty((B, 2, T, C), np.float32)
    for c in range(NC_):
        ot = res.results[c]["out_t"]                   # [P, CT, T]
        out[c // 2, c % 2] = ot.transpose(1, 0, 2).reshape(C, T).T
    return out.reshape(B, N, C)


# revision 17
# speedup vs baseline: 1.4678x; 1.0485x over previous
"""Trainium2 Bass kernel: 4-layer MoE transformer decoder (B=4,N=1024,C=1024,
H=16,D=64,HID=2048, layer0 dense GELU FFN, layers1-3 MoE E=8 top-2).

Sharding: tokens (B*N=4096) split 8 ways (512/core, core c = batch c//2 half
c%2). Weights replicated. Attention needs full-batch K/V -> one 8-core
AllGather per layer; readback uses partition-id-based dynamic DMA offsets.

Activations are kept feature-major ([C partitions, tokens free]) so every
matmul uses weights as the stationary operand. All matmuls fp32: the MoE gate
top-2 margins go down to 2.6e-6, so reduced-precision matmuls upstream of any
gate flip token routing vs the fp32 reference and blow the absmax error.
"""
import os, sys, types

sys.path.insert(0, "/opt/trn_rl_repo")
try:  # profiling hook (missing module in this image); harmless if absent
    from trn_agent_boot.trn_boot import _ntff_profile_via_ctypes
    if 'antenv.axon_hooks' not in sys.modules:
        _m = types.ModuleType('antenv.axon_hooks')
        _m.get_axon_ntff_profile_hook = (
            lambda: _ntff_profile_via_ctypes('/opt/axon/libaxon_pjrt.so'))
        sys.modules['antenv.axon_hooks'] = _m
except Exception:
    pass

import numpy as np
import concourse.bass as bass
import concourse.tile as tile
from concourse import bacc, mybir
from concourse.bass_utils import run_bass_kernel_spmd
from concourse.masks import make_identity

F32 = mybir.dt.float32
BF16 = mybir.dt.bfloat16
AF = mybir.ActivationFunctionType
OP = mybir.AluOpType

B, N, C = 4, 1024, 1024
H, D = 16, 64
HID = 2048
L, NDENSE = 4, 1
E, TOPK = 8, 2
NC_ = 8              # cores
T = 512              # tokens per core
P = 128
CT = C // P          # 8 c-tiles
HT = HID // P        # 16 hid-tiles
EPS = 1e-5

LAST_RESULT = None   # test.py reads exec_time_ns from here


def _build():
    nc = bacc.Bacc("TRN2", target_bir_lowering=False, debug=False,
                   num_devices=NC_)
    dp = nc.declare_dram_parameter
    xt_in = dp("xt", [P, CT, T], F32, isOutput=False)
    out_d = dp("out_t", [P, CT, T], F32, isOutput=True)
    W = {}
    for l in range(L):
        W[f"l{l}_ln1g"] = dp(f"l{l}_ln1g", [P, CT], F32, isOutput=False)
        W[f"l{l}_ln1b"] = dp(f"l{l}_ln1b", [P, CT], F32, isOutput=False)
        W[f"l{l}_ln2g"] = dp(f"l{l}_ln2g", [P, CT], F32, isOutput=False)
        W[f"l{l}_ln2b"] = dp(f"l{l}_ln2b", [P, CT], F32, isOutput=False)
        W[f"l{l}_qkvw"] = dp(f"l{l}_qkvw", [P, CT, 2, 3 * H * D], BF16, isOutput=False)
        W[f"l{l}_outw"] = dp(f"l{l}_outw", [P, CT, 2, C], BF16, isOutput=False)
        W[f"l{l}_outb"] = dp(f"l{l}_outb", [P, CT], F32, isOutput=False)
        if l < NDENSE:
            W[f"l{l}_ffw1"] = dp(f"l{l}_ffw1", [P, CT, 2, HID], BF16, isOutput=False)
            W[f"l{l}_ffb1"] = dp(f"l{l}_ffb1", [P, HT], F32, isOutput=False)
            W[f"l{l}_ffw2"] = dp(f"l{l}_ffw2", [P, HT, 2, C], BF16, isOutput=False)
            W[f"l{l}_ffb2"] = dp(f"l{l}_ffb2", [P, CT], F32, isOutput=False)
        else:
            W[f"l{l}_gwT"] = dp(f"l{l}_gwT", [P, CT, E], F32, isOutput=False)
            if l < L - 1:   # hi/lo bf16 pairs (3-term compensated matmuls)
                W[f"l{l}_exw1"] = dp(f"l{l}_exw1", [E, P, CT, 2, HID], BF16, isOutput=False)
                W[f"l{l}_exw3"] = dp(f"l{l}_exw3", [E, P, CT, 2, HID], BF16, isOutput=False)
                W[f"l{l}_exw2"] = dp(f"l{l}_exw2", [E, P, HT, 2, C], BF16, isOutput=False)
            else:           # last layer feeds no gate: plain bf16
                W[f"l{l}_exw1"] = dp(f"l{l}_exw1", [E, P, CT, HID], BF16, isOutput=False)
                W[f"l{l}_exw3"] = dp(f"l{l}_exw3", [E, P, CT, HID], BF16, isOutput=False)
                W[f"l{l}_exw2"] = dp(f"l{l}_exw2", [E, P, HT, C], BF16, isOutput=False)
            W[f"l{l}_exb1"] = dp(f"l{l}_exb1", [E, P, HT], F32, isOutput=False)
            W[f"l{l}_exb3"] = dp(f"l{l}_exb3", [E, P, HT], F32, isOutput=False)
            W[f"l{l}_exb2r"] = dp(f"l{l}_exb2r", [E, CT, P], F32, isOutput=False)

    with tile.TileContext(nc) as tc:
        _emit(nc, tc, xt_in, out_d, W)
    nc.compile()
    return nc


def _emit(nc, tc, xt_in, out_d, W):
    from contextlib import ExitStack
    ctx = ExitStack()
    const = ctx.enter_context(tc.tile_pool(name="const", bufs=1))
    big = ctx.enter_context(tc.tile_pool(name="big", bufs=1))
    wp = ctx.enter_context(tc.tile_pool(name="wp", bufs=2))
    ptp = ctx.enter_context(tc.tile_pool(name="ptp", bufs=3))
    sm = ctx.enter_context(tc.tile_pool(name="sm", bufs=1))
    sm2 = ctx.enter_context(tc.tile_pool(name="sm2", bufs=2))
    dram = ctx.enter_context(tc.tile_pool(name="dram", bufs=1, space="DRAM"))
    ps_mm = ctx.enter_context(tc.tile_pool(name="ps_mm", bufs=2, space="PSUM"))
    ps_aux = ctx.enter_context(tc.tile_pool(name="ps_aux", bufs=2, space="PSUM"))
    ps_av = ctx.enter_context(tc.tile_pool(name="ps_av", bufs=2, space="PSUM"))
    ps_st = ctx.enter_context(tc.tile_pool(name="ps_st", bufs=2, space="PSUM"))

    ident = const.tile([P, P], F32)
    make_identity(nc, ident[:])
    ones_c = const.tile([P, 1], F32)      # column of ones (lhsT for col-sums)
    nc.vector.memset(ones_c[:], 1.0)
    ones_r = const.tile([1, P], F32)      # row of ones (lhsT for broadcasts)
    nc.vector.memset(ones_r[:], 1.0)

    # residual, feature-major [P, CT, T]
    resid = const.tile([P, CT, T], F32)
    nc.sync.dma_start(out=resid[:], in_=xt_in[:])

    pid = nc.sync.partition_id()
    pair_base = (pid // 2) * 2 * P        # AG-row offset of my batch pair

    def layer_norm(l, which, dst_tag):
        """LN over the feature (partition x ctile) axis; returns h tile."""
        g = sm.tile([P, CT], F32, tag="lng", name=f"g{l}{which}")
        b = sm.tile([P, CT], F32, tag="lnb", name=f"b{l}{which}")
        nc.sync.dma_start(out=g[:], in_=W[f"l{l}_ln{which}g"][:])
        nc.sync.dma_start(out=b[:], in_=W[f"l{l}_ln{which}b"][:])
        sq = big.tile([P, CT, T], F32, tag="tmp2m", name=f"sq{l}{which}")
        for ct in range(CT):
            nc.vector.tensor_mul(out=sq[:, ct, :], in0=resid[:, ct, :],
                                 in1=resid[:, ct, :])
        ps1 = ps_st.tile([1, T], F32, tag="st", name=f"ps1_{l}{which}")
        ps2 = ps_st.tile([1, T], F32, tag="st", name=f"ps2_{l}{which}")
        for ct in range(CT):
            nc.tensor.matmul(out=ps1[:], lhsT=ones_c[:], rhs=resid[:, ct, :],
                             start=(ct == 0), stop=(ct == CT - 1))
        for ct in range(CT):
            nc.tensor.matmul(out=ps2[:], lhsT=ones_c[:], rhs=sq[:, ct, :],
                             start=(ct == 0), stop=(ct == CT - 1))
        st = sm.tile([1, 6, T], F32, tag="stats", name=f"st{l}{which}")
        mu, msq, varp, rinv, rstd, numu = (st[:, i, :] for i in range(6))
        nc.vector.tensor_scalar_mul(out=mu, in0=ps1[:], scalar1=1.0 / C)
        nc.vector.tensor_scalar_mul(out=msq, in0=ps2[:], scalar1=1.0 / C)
        nc.vector.tensor_mul(out=varp, in0=mu, in1=mu)
        nc.vector.tensor_tensor(out=varp, in0=msq, in1=varp, op=OP.subtract)
        nc.vector.tensor_scalar_add(out=varp, in0=varp, scalar1=EPS)
        sd = st[:, 1, :]  # reuse msq slot
        nc.scalar.activation(out=sd, in_=varp, func=AF.Sqrt)
        nc.vector.reciprocal(out=rinv, in_=sd)
        # one Newton step: r = rinv*(1.5 - 0.5*varp*rinv^2)
        nc.vector.tensor_mul(out=sd, in0=rinv, in1=rinv)
        nc.vector.tensor_mul(out=sd, in0=sd, in1=varp)
        nc.vector.tensor_scalar(out=sd, in0=sd, scalar1=-0.5, scalar2=1.5,
                                op0=OP.mult, op1=OP.add)
        nc.vector.tensor_mul(out=rstd, in0=rinv, in1=sd)
        nc.vector.tensor_mul(out=numu, in0=mu, in1=rstd)
        nc.vector.tensor_scalar_mul(out=numu, in0=numu, scalar1=-1.0)
        psR = ps_aux.tile([P, T], F32, tag="aux", name=f"psR{l}{which}")
        psM = ps_aux.tile([P, T], F32, tag="aux", name=f"psM{l}{which}")
        nc.tensor.matmul(out=psR[:], lhsT=ones_r[:], rhs=rstd, start=True, stop=True)
        nc.tensor.matmul(out=psM[:], lhsT=ones_r[:], rhs=numu, start=True, stop=True)
        h = big.tile([P, CT, T], F32, tag=dst_tag, name=f"h{l}{which}")
        for ct in range(CT):
            nc.vector.tensor_tensor(out=h[:, ct, :], in0=resid[:, ct, :],
                                    in1=psR[:], op=OP.mult)
            nc.vector.tensor_tensor(out=h[:, ct, :], in0=h[:, ct, :],
                                    in1=psM[:], op=OP.add)
            nc.vector.tensor_scalar(out=h[:, ct, :], in0=h[:, ct, :],
                                    scalar1=g[:, ct:ct + 1], scalar2=b[:, ct:ct + 1],
                                    op0=OP.mult, op1=OP.add)
        return h

    def split_pair(srcf32, kt, tag, nm):
        """f32 [P, kt, T] -> bf16 hi/lo pair [P, 2, kt, T]"""
        hp = big.tile([P, 2, kt, T], BF16, tag=tag, name=f"sp{nm}")
        for k in range(kt):
            nc.vector.tensor_copy(out=hp[:, 0, k, :], in_=srcf32[:, k, :])
            nc.vector.tensor_tensor(out=hp[:, 1, k, :], in0=srcf32[:, k, :],
                                    in1=hp[:, 0, k, :], op=OP.subtract)
        return hp

    def matmul_block(dst, dst_slice_fn, w_dram, hp, kt, n_cols, l, nm,
                     act=None, bias=None, chunk_cols=512):
        """dst[.., n] = act(w.T @ h + bias); w_dram [P, kt, 2, n_cols] bf16
        hi/lo pairs, hp [P, 2, kt, T] bf16 hi/lo; 3-term compensated."""
        for c0 in range(0, n_cols, chunk_cols):
            cw = min(chunk_cols, n_cols - c0)
            wc = wp.tile([P, kt, 2, chunk_cols], BF16, tag="w", name=f"w{nm}_{c0}")
            nc.sync.dma_start(out=wc[:, :, :, :cw], in_=w_dram[:, :, :, c0:c0 + cw])
            for ni in range(0, cw, P):
                psx = ps_mm.tile([P, T], F32, tag="mm", name=f"ps{nm}_{c0}_{ni}")
                for k in range(kt):
                    wh, wl = wc[:, k, 0, ni:ni + P], wc[:, k, 1, ni:ni + P]
                    nc.tensor.matmul(out=psx[:], lhsT=wh, rhs=hp[:, 0, k, :],
                                     start=(k == 0), stop=False)
                    nc.tensor.matmul(out=psx[:], lhsT=wh, rhs=hp[:, 1, k, :],
                                     start=False, stop=False)
                    nc.tensor.matmul(out=psx[:], lhsT=wl, rhs=hp[:, 0, k, :],
                                     start=False, stop=(k == kt - 1))
                n_idx = (c0 + ni) // P
                dslice = dst_slice_fn(dst, n_idx)
                if act is not None:
                    bb = bias[:, n_idx:n_idx + 1] if bias is not None else 0.0
                    nc.scalar.activation(out=dslice, in_=psx[:], func=act, bias=bb)
                elif bias is not None:
                    nc.vector.tensor_scalar_add(out=dslice, in0=psx[:],
                                                scalar1=bias[:, n_idx:n_idx + 1])
                else:
                    nc.vector.tensor_copy(out=dslice, in_=psx[:])

    for l in range(L):
        # ---- LN1 + attention ----
        h = layer_norm(l, 1, "h")
        h1p = split_pair(h, CT, "tmp2m", f"h1p{l}")
        qkv_d = W[f"l{l}_qkvw"]
        # keys stored MY-HALF-FIRST (softmax is key-permutation invariant):
        # kTf[:, :, 0:512] = my keys (written straight from the k-matmuls),
        # kTf[:, :, 512:1024] = partner's (arrives via AllGather readback).
        kTf = big.tile([P, CT, 1024], F32, tag="big4a", name=f"kTf{l}")
        vaug = big.tile([P, 8, H, 65], F32, tag="big4b", name=f"vaug{l}")
        nc.vector.memset(vaug[:, :, :, 64:65], 1.0)
        matmul_block(kTf, lambda d, n: d[:, n, 0:512], qkv_d[:, :, :, 1024:2048],
                     h1p, CT, 1024, l, f"k{l}")
        # v token-major into vaug chunks 0..3; lhsT = h token-chunks (hi/lo)
        for nv in range(2):
            wc = wp.tile([P, CT, 2, 512], BF16, tag="w", name=f"wv{l}_{nv}")
            nc.sync.dma_start(out=wc[:],
                              in_=qkv_d[:, :, :, 2048 + nv * 512: 2048 + (nv + 1) * 512])
            for m in range(4):
                psv = ps_mm.tile([P, T], F32, tag="mm", name=f"psv{l}_{m}_{nv}")
                for k in range(CT):
                    lh = h1p[:, 0, k, m * P:(m + 1) * P]
                    ll = h1p[:, 1, k, m * P:(m + 1) * P]
                    nc.tensor.matmul(out=psv[:], lhsT=lh, rhs=wc[:, k, 0, :],
                                     start=(k == 0), stop=False)
                    nc.tensor.matmul(out=psv[:], lhsT=ll, rhs=wc[:, k, 0, :],
                                     start=False, stop=False)
                    nc.tensor.matmul(out=psv[:], lhsT=lh, rhs=wc[:, k, 1, :],
                                     start=False, stop=(k == CT - 1))
                nc.vector.tensor_copy(
                    out=vaug[:, m, nv * 8:(nv + 1) * 8, 0:64], in_=psv[:])
        # bounce my kT + v to DRAM and kick the AllGather NOW; the q-matmuls
        # and my-half attention below overlap with it on PE.
        cin = dram.tile([P, 8192], F32, tag="cin", name=f"cin{l}")
        cout = dram.tile([NC_ * P, 8192], F32, tag="cout", name=f"cout{l}")
        nc.sync.dma_start(out=cin[:, 0:4096].rearrange("p (c t) -> p c t", c=CT),
                          in_=kTf[:, :, 0:512])
        nc.sync.dma_start(
            out=cin[:, 4096:8192].rearrange("p (c h d) -> p c h d", c=4, h=H),
            in_=vaug[:, 0:4, :, 0:64])
        nc.gpsimd.collective_compute(
            "AllGather", OP.bypass, replica_groups=[list(range(NC_))],
            ins=[cin.opt()], outs=[cout.opt()])
        qT = big.tile([P, CT, T], F32, tag="qT", name=f"qT{l}")
        matmul_block(qT, lambda d, n: d[:, n, :], qkv_d[:, :, :, 0:1024], h1p,
                     CT, 1024, l, f"q{l}")
        # my-half attention for heads 0..7 (no partner data needed); the psA
        # partial is copied out so the psum bank frees immediately
        av_pack = big.tile([65, 8, T], F32, tag="h", name=f"avp{l}")
        for hd in range(8):
            hp, half = hd // 2, hd % 2
            rows = slice(half * 64, half * 64 + 64)
            psA = ps_av.tile([65, T], F32, tag="av", name=f"psA{l}_{hd}a")
            for kc in range(4):
                psS = ps_aux.tile([P, T], F32, tag="aux", name=f"psS{l}_{hd}_{kc}")
                nc.tensor.matmul(out=psS[:], lhsT=kTf[rows, hp, kc * P:(kc + 1) * P],
                                 rhs=qT[rows, hp, :], start=True, stop=True)
                pt = ptp.tile([P, T], F32, tag="pt", name=f"pt{l}_{hd}_{kc}")
                nc.scalar.activation(out=pt[:], in_=psS[:], func=AF.Exp, scale=0.125)
                nc.tensor.matmul(out=psA[:], lhsT=vaug[:, kc, hd, :], rhs=pt[:],
                                 start=(kc == 0), stop=(kc == 3))
            nc.vector.tensor_copy(out=av_pack[:, hd, :], in_=psA[:])
        # partner readback: row (pid^1) of the AllGather output
        parity = pid - (pid // 2) * 2
        partner = nc.s_assert_within(pid + 1 - parity * 2, 0, NC_ - 1, skip_runtime_assert=True)
        srcp = cout[bass.ds(partner * P, P), :]
        nc.sync.dma_start(
            out=kTf[:, :, 512:1024],
            in_=srcp[:, 0:4096].rearrange("p (c t) -> p c t", c=CT))
        nc.sync.dma_start(
            out=vaug[:, 4:8, :, 0:64],
            in_=srcp[:, 4096:8192].rearrange("p (c h d) -> p c h d", c=4, h=H))
        attT = big.tile([P, CT, T], F32, tag="tmp2m", name=f"attT{l}")
        for hd in range(H):
            hp, half = hd // 2, hd % 2
            rows = slice(half * 64, half * 64 + 64)
            two_pass = hd < 8
            kc0 = 4 if two_pass else 0
            psA = ps_av.tile([65, T], F32, tag="av", name=f"psA{l}_{hd}b")
            for kc in range(kc0, 8):
                psS = ps_aux.tile([P, T], F32, tag="aux", name=f"psS{l}_{hd}_{kc}b")
                nc.tensor.matmul(out=psS[:], lhsT=kTf[rows, hp, kc * P:(kc + 1) * P],
                                 rhs=qT[rows, hp, :], start=True, stop=True)
                pt = ptp.tile([P, T], F32, tag="pt", name=f"pt{l}_{hd}_{kc}b")
                nc.scalar.activation(out=pt[:], in_=psS[:], func=AF.Exp, scale=0.125)
                nc.tensor.matmul(out=psA[:], lhsT=vaug[:, kc, hd, :], rhs=pt[:],
                                 start=(kc == kc0), stop=(kc == 7))
            av = sm2.tile([65, T], F32, tag="avs", name=f"av{l}_{hd}")
            if two_pass:
                nc.vector.tensor_tensor(out=av[:], in0=av_pack[:, hd, :],
                                        in1=psA[:], op=OP.add)
            else:
                nc.vector.tensor_copy(out=av[:], in_=psA[:])
            rec = sm2.tile([1, T], F32, tag="rec", name=f"rec{l}_{hd}")
            nc.vector.reciprocal(out=rec[:], in_=av[64:65, :])
            psB = ps_aux.tile([64, T], F32, tag="aux", name=f"psB{l}_{hd}")
            nc.tensor.matmul(out=psB[:], lhsT=ones_r[:, 0:64], rhs=rec[:],
                             start=True, stop=True)
            nc.vector.tensor_tensor(out=attT[rows, hp, :], in0=av[0:64, :],
                                    in1=psB[:], op=OP.mult)
        # out-projection + residual add
        outb = sm.tile([P, CT], F32, tag="lnb2", name=f"outb{l}")
        nc.sync.dma_start(out=outb[:], in_=W[f"l{l}_outb"][:])
        attTp = split_pair(attT, CT, "h", f"attTp{l}")
        ow_d = W[f"l{l}_outw"]
        for c0 in (0, 512):
            wc = wp.tile([P, CT, 2, 512], BF16, tag="w", name=f"wo{l}_{c0}")
            nc.sync.dma_start(out=wc[:], in_=ow_d[:, :, :, c0:c0 + 512])
            for ni in range(4):
                ct = (c0 + ni * P) // P
                psx = ps_mm.tile([P, T], F32, tag="mm", name=f"pso{l}_{ct}")
                for k in range(CT):
                    wh, wl = wc[:, k, 0, ni * P:(ni + 1) * P], wc[:, k, 1, ni * P:(ni + 1) * P]
                    nc.tensor.matmul(out=psx[:], lhsT=wh, rhs=attTp[:, 0, k, :],
                                     start=(k == 0), stop=False)
                    nc.tensor.matmul(out=psx[:], lhsT=wh, rhs=attTp[:, 1, k, :],
                                     start=False, stop=False)
                    nc.tensor.matmul(out=psx[:], lhsT=wl, rhs=attTp[:, 0, k, :],
                                     start=False, stop=(k == CT - 1))
                tb = sm2.tile([P, T], F32, tag="projtmp", name=f"tb{l}_{ct}")
                nc.vector.tensor_scalar_add(out=tb[:], in0=psx[:],
                                            scalar1=outb[:, ct:ct + 1])
                nc.vector.tensor_tensor(out=resid[:, ct, :], in0=resid[:, ct, :],
                                        in1=tb[:], op=OP.add)

        # ---- LN2 + FFN/MoE ----
        h2 = layer_norm(l, 2, "h")
        if l < NDENSE:
            ffb1 = sm.tile([P, HT], F32, tag="lnb2", name=f"ffb1{l}")
            nc.sync.dma_start(out=ffb1[:], in_=W[f"l{l}_ffb1"][:])
            h2p = split_pair(h2, CT, "tmp2m", f"h2pd{l}")
            f1 = big.tile([P, HT, T], F32, tag="big4a", name=f"f1{l}")
            matmul_block(f1, lambda d, n: d[:, n, :], W[f"l{l}_ffw1"], h2p, CT,
                         HID, l, f"ff1{l}", act=AF.Gelu, bias=ffb1)
            ffb2 = sm.tile([P, CT], F32, tag="lnb2", name=f"ffb2{l}")
            nc.sync.dma_start(out=ffb2[:], in_=W[f"l{l}_ffb2"][:])
            f1p = split_pair(f1, HT, "big4b", f"f1p{l}")
            f2w = W[f"l{l}_ffw2"]
            for ct in range(CT):
                psx = ps_mm.tile([P, T], F32, tag="mm", name=f"psf2{l}_{ct}")
                for kh in range(2):
                    wc = wp.tile([P, CT, 2, P], BF16, tag="w", name=f"wf2{l}_{ct}_{kh}")
                    nc.sync.dma_start(
                        out=wc[:],
                        in_=f2w[:, kh * CT:(kh + 1) * CT, :, ct * P:(ct + 1) * P])
                    for k in range(CT):
                        kk = kh * CT + k
                        wh, wl = wc[:, k, 0, :], wc[:, k, 1, :]
                        nc.tensor.matmul(out=psx[:], lhsT=wh, rhs=f1p[:, 0, kk, :],
                                         start=(kk == 0), stop=False)
                        nc.tensor.matmul(out=psx[:], lhsT=wh, rhs=f1p[:, 1, kk, :],
                                         start=False, stop=False)
                        nc.tensor.matmul(out=psx[:], lhsT=wl, rhs=f1p[:, 0, kk, :],
                                         start=False, stop=(kk == HT - 1))
                tb = sm2.tile([P, T], F32, tag="projtmp", name=f"tf{l}_{ct}")
                nc.vector.tensor_scalar_add(out=tb[:], in0=psx[:],
                                            scalar1=ffb2[:, ct:ct + 1])
                nc.vector.tensor_tensor(out=resid[:, ct, :], in0=resid[:, ct, :],
                                        in1=tb[:], op=OP.add)
        else:
            _moe(nc, tc, l, W, h2, resid, ident, ones_r, big, wp, sm, sm2,
                 ps_mm, ps_aux, ps_st)

    nc.sync.dma_start(out=out_d[:], in_=resid[:])
    ctx.close()


def _moe(nc, tc, l, W, h2, resid, ident, ones_r, big, wp, sm, sm2, ps_mm, ps_aux, ps_st):
    # gate logits [E, T] feature-major
    gw = sm.tile([P, CT, E], F32, tag="gw", name=f"gw{l}")
    nc.sync.dma_start(out=gw[:], in_=W[f"l{l}_gwT"][:])
    psg = ps_st.tile([E, T], F32, tag="st", name=f"psg{l}")
    for k in range(CT):
        nc.tensor.matmul(out=psg[:], lhsT=gw[:, k, :], rhs=h2[:, k, :],
                         start=(k == 0), stop=(k == CT - 1))
    lg = sm.tile([E, T], F32, tag="lg", name=f"lg{l}")
    nc.vector.tensor_copy(out=lg[:], in_=psg[:])
    # transpose to token-major [128, 4, E]
    lgT = sm.tile([P, 4, E], F32, tag="lgT", name=f"lgT{l}")
    for j in range(4):
        pst = ps_st.tile([P, E], F32, tag="st", name=f"pst{l}_{j}")
        nc.tensor.transpose(out=pst[:], in_=lg[:, j * P:(j + 1) * P],
                            identity=ident[0:E, 0:E])
        nc.vector.tensor_copy(out=lgT[:, j, :], in_=pst[:])
    # top-2 mask + softmax (max-subtracted, matching reference)
    wk = sm.tile([P, 4, 6, E], F32, tag="wk", name=f"wk{l}")
    m1 = sm.tile([P, 4, 4], F32, tag="m1", name=f"m1{l}")
    for j in range(4):
        nc.vector.tensor_reduce(out=m1[:, j, 0:1], in_=lgT[:, j, :],
                                axis=mybir.AxisListType.X, op=OP.max)
        # eq-mask of the max, knock it out, then second max
        nc.vector.tensor_scalar(out=wk[:, j, 0, :], in0=lgT[:, j, :],
                                scalar1=m1[:, j, 0:1], scalar2=None,
                                op0=OP.is_equal)
        nc.vector.tensor_scalar_mul(out=wk[:, j, 1, :], in0=wk[:, j, 0, :],
                                    scalar1=1e30)
        nc.vector.tensor_tensor(out=wk[:, j, 1, :], in0=lgT[:, j, :],
                                in1=wk[:, j, 1, :], op=OP.subtract)
        nc.vector.tensor_reduce(out=m1[:, j, 1:2], in_=wk[:, j, 1, :],
                                axis=mybir.AxisListType.X, op=OP.max)
        nc.vector.tensor_scalar(out=wk[:, j, 2, :], in0=lgT[:, j, :],
                                scalar1=m1[:, j, 1:2], scalar2=None,
                                op0=OP.is_ge)
        # softmax exp(x - max)
        nc.vector.tensor_scalar_mul(out=m1[:, j, 2:3], in0=m1[:, j, 0:1],
                                    scalar1=-1.0)
        nc.scalar.activation(out=wk[:, j, 3, :], in_=lgT[:, j, :], func=AF.Exp,
                             bias=m1[:, j, 2:3])
        nc.vector.tensor_reduce(out=m1[:, j, 3:4], in_=wk[:, j, 3, :],
                                axis=mybir.AxisListType.X, op=OP.add)
        nc.vector.reciprocal(out=m1[:, j, 3:4], in_=m1[:, j, 3:4])
        nc.vector.tensor_mul(out=wk[:, j, 4, :], in0=wk[:, j, 3, :],
                             in1=wk[:, j, 2, :])
        nc.vector.tensor_scalar_mul(out=wk[:, j, 5, :], in0=wk[:, j, 4, :],
                                    scalar1=m1[:, j, 3:4])
    # weT [E, T] feature-major combine weights
    weT = sm.tile([E, T], F32, tag="lg2", name=f"weT{l}")
    for j in range(4):
        pst = ps_st.tile([E, P], F32, tag="st", name=f"psu{l}_{j}")
        nc.tensor.transpose(out=pst[:], in_=wk[:, j, 5, :], identity=ident[:])
        nc.vector.tensor_copy(out=weT[:, j * P:(j + 1) * P], in_=pst[:])

    moe_out = big.tile([P, CT, T], F32, tag="qT", name=f"moeout{l}")
    b2mode = (l < L - 1)
    # split h2 into bf16 hi (+ lo for the compensated layers)
    h2p = big.tile([P, 2, CT, T], BF16, tag="tmp2m", name=f"h2p{l}")
    for ct in range(CT):
        nc.vector.tensor_copy(out=h2p[:, 0, ct, :], in_=h2[:, ct, :])
        if b2mode:
            nc.vector.tensor_tensor(out=h2p[:, 1, ct, :], in0=h2[:, ct, :],
                                    in1=h2p[:, 0, ct, :], op=OP.subtract)

    def mm_terms(psx, wh, wl, rh, rl, k, kt):
        n_terms = 3 if b2mode else 1
        first = (k == 0)
        last = (k == kt - 1)
        nc.tensor.matmul(out=psx[:], lhsT=wh, rhs=rh, start=first,
                         stop=(last and n_terms == 1))
        if b2mode:
            nc.tensor.matmul(out=psx[:], lhsT=wh, rhs=rl, start=False, stop=False)
            nc.tensor.matmul(out=psx[:], lhsT=wl, rhs=rh, start=False, stop=last)

    for e in range(E):
        b1 = sm.tile([P, HT], F32, tag="lnb2", name=f"exb1{l}_{e}")
        b3 = sm.tile([P, HT], F32, tag="lnb3", name=f"exb3{l}_{e}")
        nc.sync.dma_start(out=b1[:], in_=W[f"l{l}_exb1"][e])
        nc.sync.dma_start(out=b3[:], in_=W[f"l{l}_exb3"][e])
        g1 = big.tile([P, HT, T], F32, tag="big4a", name=f"g1_{l}_{e}")
        g3 = big.tile([P, HT, T], F32, tag="big4b", name=f"g3_{l}_{e}")
        w1d, w3d, w2d = (W[f"l{l}_exw1"][e], W[f"l{l}_exw3"][e], W[f"l{l}_exw2"][e])
        for c0 in range(0, HID, 512):
            for (wd, gg, bb, acts) in ((w1d, g1, b1, AF.Silu), (w3d, g3, b3, None)):
                if b2mode:
                    wc = wp.tile([P, CT, 2, 512], BF16, tag="w",
                                 name=f"we{l}_{e}_{c0}_{acts}")
                    nc.sync.dma_start(out=wc[:], in_=wd[:, :, :, c0:c0 + 512])
                else:
                    wc = wp.tile([P, CT, 512], BF16, tag="w",
                                 name=f"we{l}_{e}_{c0}_{acts}")
                    nc.sync.dma_start(out=wc[:], in_=wd[:, :, c0:c0 + 512])
                for ni in range(4):
                    nt = (c0 + ni * P) // P
                    psx = ps_mm.tile([P, T], F32, tag="mm",
                                     name=f"pse{l}_{e}_{nt}_{acts}")
                    for k in range(CT):
                        if b2mode:
                            wh, wl = wc[:, k, 0, ni * P:(ni + 1) * P], wc[:, k, 1, ni * P:(ni + 1) * P]
                        else:
                            wh = wl = wc[:, k, ni * P:(ni + 1) * P]
                        mm_terms(psx, wh, wl, h2p[:, 0, k, :], h2p[:, 1, k, :], k, CT)
                    if acts is not None:
                        nc.scalar.activation(out=gg[:, nt, :], in_=psx[:],
                                             func=acts, bias=bb[:, nt:nt + 1])
                    else:
                        nc.vector.tensor_scalar_add(out=gg[:, nt, :], in0=psx[:],
                                                    scalar1=bb[:, nt:nt + 1])
        # we_e broadcast (K=1 matmul; row DMA'd to partition 0 first)
        werow = sm2.tile([1, T], F32, tag="werow", name=f"werow{l}_{e}")
        nc.sync.dma_start(out=werow[:], in_=weT[e:e + 1, :])
        psW = ps_aux.tile([P, T], F32, tag="aux", name=f"psW{l}_{e}")
        nc.tensor.matmul(out=psW[:], lhsT=ones_r[:], rhs=werow[:],
                         start=True, stop=True)
        # ge = silu(g1)*g3*we -> f32 in g3, then bf16 hi/lo for the w2 matmul
        for k in range(HT):
            nc.vector.tensor_tensor(out=g1[:, k, :], in0=g1[:, k, :],
                                    in1=g3[:, k, :], op=OP.mult)
            nc.vector.tensor_tensor(out=g3[:, k, :], in0=g1[:, k, :],
                                    in1=psW[:], op=OP.mult)
        gehl = big.tile([P, 2, HT, T], BF16, tag="big4a", name=f"gehl{l}_{e}")
        for k in range(HT):
            nc.vector.tensor_copy(out=gehl[:, 0, k, :], in_=g3[:, k, :])
            if b2mode:
                nc.vector.tensor_tensor(out=gehl[:, 1, k, :], in0=g3[:, k, :],
                                        in1=gehl[:, 0, k, :], op=OP.subtract)
        # w2: [HID -> C], accumulate over experts in moe_out
        for ct in range(CT):
            psx = ps_mm.tile([P, T], F32, tag="mm", name=f"ps2{l}_{e}_{ct}")
            for kh in range(2):
                if b2mode:
                    wc = wp.tile([P, CT, 2, P], BF16, tag="w",
                                 name=f"w2{l}_{e}_{ct}_{kh}")
                    nc.sync.dma_start(
                        out=wc[:],
                        in_=w2d[:, kh * CT:(kh + 1) * CT, :, ct * P:(ct + 1) * P])
                else:
                    wc = wp.tile([P, CT, P], BF16, tag="w",
                                 name=f"w2{l}_{e}_{ct}_{kh}")
                    nc.sync.dma_start(
                        out=wc[:],
                        in_=w2d[:, kh * CT:(kh + 1) * CT, ct * P:(ct + 1) * P])
                for k in range(CT):
                    if b2mode:
                        wh, wl = wc[:, k, 0, :], wc[:, k, 1, :]
                    else:
                        wh = wl = wc[:, k, :]
                    kk = kh * CT + k
                    mm_terms(psx, wh, wl, gehl[:, 0, kk, :], gehl[:, 1, kk, :],
                             kk, HT)
            if e == 0:
                nc.vector.tensor_copy(out=moe_out[:, ct, :], in_=psx[:])
            else:
                nc.vector.tensor_tensor(out=moe_out[:, ct, :],
                                        in0=moe_out[:, ct, :], in1=psx[:],
                                        op=OP.add)
    # bias contribution: sum_e we_e * b2_e  == exb2r.T @ weT  (contraction E)
    b2r = sm.tile([E, CT, P], F32, tag="b2r", name=f"b2r{l}")
    nc.sync.dma_start(out=b2r[:], in_=W[f"l{l}_exb2r"][:])
    for ct in range(CT):
        psx = ps_aux.tile([P, T], F32, tag="aux", name=f"psb2{l}_{ct}")
        nc.tensor.matmul(out=psx[:], lhsT=b2r[:, ct, :], rhs=weT[:],
                         start=True, stop=True)
        nc.vector.tensor_tensor(out=moe_out[:, ct, :], in0=moe_out[:, ct, :],
                                in1=psx[:], op=OP.add)
        nc.vector.tensor_tensor(out=resid[:, ct, :], in0=resid[:, ct, :],
                                in1=moe_out[:, ct, :], op=OP.add)


def _stage_weights(params):
    fl = {}
    for l, p in enumerate(params):
        g = lambda k: np.ascontiguousarray(np.asarray(p[k], dtype=np.float32))
        fl[f"l{l}_ln1g"] = g("ln1_g").reshape(CT, P).T.copy()
        fl[f"l{l}_ln1b"] = g("ln1_b").reshape(CT, P).T.copy()
        fl[f"l{l}_ln2g"] = g("ln2_g").reshape(CT, P).T.copy()
        fl[f"l{l}_ln2b"] = g("ln2_b").reshape(CT, P).T.copy()
        import ml_dtypes
        _bf = ml_dtypes.bfloat16
        def split2(w):  # [P, kt, n] f32 -> [P, kt, 2, n] bf16 hi/lo
            wh = w.astype(_bf)
            wl = (w - wh.astype(np.float32)).astype(_bf)
            return np.ascontiguousarray(np.stack([wh, wl], axis=2))
        fl[f"l{l}_qkvw"] = split2(g("qkv_w").reshape(CT, P, 3 * H * D).transpose(1, 0, 2))
        fl[f"l{l}_outw"] = split2(g("out_w").reshape(CT, P, C).transpose(1, 0, 2))
        fl[f"l{l}_outb"] = g("out_b").reshape(CT, P).T.copy()
        if "ff_w1" in p:
            fl[f"l{l}_ffw1"] = split2(g("ff_w1").reshape(CT, P, HID).transpose(1, 0, 2))
            fl[f"l{l}_ffb1"] = g("ff_b1").reshape(HT, P).T.copy()
            fl[f"l{l}_ffw2"] = split2(g("ff_w2").reshape(HT, P, C).transpose(1, 0, 2))
            fl[f"l{l}_ffb2"] = g("ff_b2").reshape(CT, P).T.copy()
        else:
            fl[f"l{l}_gwT"] = np.ascontiguousarray(g("gate_w").T).reshape(CT, P, E).transpose(1, 0, 2).copy()
            import ml_dtypes
            bf = ml_dtypes.bfloat16
            w1 = g("ex_w1").reshape(E, CT, P, HID).transpose(0, 2, 1, 3)
            w3 = g("ex_w3").reshape(E, CT, P, HID).transpose(0, 2, 1, 3)
            w2 = g("ex_w2").reshape(E, HT, P, C).transpose(0, 2, 1, 3)
            if l < L - 1:
                def split(w):
                    wh = w.astype(bf)
                    wl = (w - wh.astype(np.float32)).astype(bf)
                    return np.ascontiguousarray(np.stack([wh, wl], axis=3))
                fl[f"l{l}_exw1"] = split(w1)
                fl[f"l{l}_exw3"] = split(w3)
                fl[f"l{l}_exw2"] = split(w2)
            else:
                fl[f"l{l}_exw1"] = np.ascontiguousarray(w1.astype(bf))
                fl[f"l{l}_exw3"] = np.ascontiguousarray(w3.astype(bf))
                fl[f"l{l}_exw2"] = np.ascontiguousarray(w2.astype(bf))
            fl[f"l{l}_exb1"] = g("ex_b1").reshape(E, HT, P).transpose(0, 2, 1).copy()
            fl[f"l{l}_exb3"] = g("ex_b3").reshape(E, HT, P).transpose(0, 2, 1).copy()
            fl[f"l{l}_exb2r"] = g("ex_b2").reshape(E, CT, P).copy()
    return fl


_NC_CACHE = None


def kernel(x, params):
    global _NC_CACHE, LAST_RESULT
    x = np.asarray(x, dtype=np.float32)
    staged = _stage_weights(params)
    if _NC_CACHE is None:
        _NC_CACHE = _build()
    nc = _NC_CACHE
    xr = x.reshape(B, 2, T, C)
    in_maps = []
    for c in range(NC_):
        shard = xr[c // 2, c % 2]                      # [T, C]
        xt = np.ascontiguousarray(shard.T.reshape(CT, P, T).transpose(1, 0, 2))
        m = {"xt": xt}
        m.update(staged)
        in_maps.append(m)
    res = run_bass_kernel_spmd(nc, in_maps, list(range(NC_)),
                               trace=bool(os.environ.get("BASS_TRACE")))
    LAST_RESULT = res
    out = np.empty((B, 2, T, C), np.float32)
    for c in range(NC_):
        ot = res.results[c]["out_t"]                   # [P, CT, T]
        out[c // 2, c % 2] = ot.transpose(1, 0, 2).reshape(C, T).T
    return out.reshape(B, N, C)


# revision 21
# speedup vs baseline: 1.4791x; 1.0077x over previous
"""Trainium2 Bass kernel: 4-layer MoE transformer decoder (B=4,N=1024,C=1024,
H=16,D=64,HID=2048, layer0 dense GELU FFN, layers1-3 MoE E=8 top-2).

Sharding: tokens (B*N=4096) split 8 ways (512/core, core c = batch c//2 half
c%2). Weights replicated. Attention needs full-batch K/V -> one 8-core
AllGather per layer; readback uses partition-id-based dynamic DMA offsets.

Activations are kept feature-major ([C partitions, tokens free]) so every
matmul uses weights as the stationary operand. All matmuls fp32: the MoE gate
top-2 margins go down to 2.6e-6, so reduced-precision matmuls upstream of any
gate flip token routing vs the fp32 reference and blow the absmax error.
"""
import os, sys, types

sys.path.insert(0, "/opt/trn_rl_repo")
try:  # profiling hook (missing module in this image); harmless if absent
    from trn_agent_boot.trn_boot import _ntff_profile_via_ctypes
    if 'antenv.axon_hooks' not in sys.modules:
        _m = types.ModuleType('antenv.axon_hooks')
        _m.get_axon_ntff_profile_hook = (
            lambda: _ntff_profile_via_ctypes('/opt/axon/libaxon_pjrt.so'))
        sys.modules['antenv.axon_hooks'] = _m
except Exception:
    pass

import numpy as np
import concourse.bass as bass
import concourse.tile as tile
from concourse import bacc, mybir
from concourse.bass_utils import run_bass_kernel_spmd
from concourse.masks import make_identity

F32 = mybir.dt.float32
BF16 = mybir.dt.bfloat16
AF = mybir.ActivationFunctionType
OP = mybir.AluOpType

B, N, C = 4, 1024, 1024
H, D = 16, 64
HID = 2048
L, NDENSE = 4, 1
E, TOPK = 8, 2
NC_ = 8              # cores
T = 512              # tokens per core
P = 128
CT = C // P          # 8 c-tiles
HT = HID // P        # 16 hid-tiles
EPS = 1e-5

LAST_RESULT = None   # test.py reads exec_time_ns from here


def _build():
    nc = bacc.Bacc("TRN2", target_bir_lowering=False, debug=False,
                   num_devices=NC_)
    dp = nc.declare_dram_parameter
    xt_in = dp("xt", [P, CT, T], F32, isOutput=False)
    out_d = dp("out_t", [P, CT, T], F32, isOutput=True)
    W = {}
    for l in range(L):
        W[f"l{l}_ln1g"] = dp(f"l{l}_ln1g", [P, CT], F32, isOutput=False)
        W[f"l{l}_ln1b"] = dp(f"l{l}_ln1b", [P, CT], F32, isOutput=False)
        W[f"l{l}_ln2g"] = dp(f"l{l}_ln2g", [P, CT], F32, isOutput=False)
        W[f"l{l}_ln2b"] = dp(f"l{l}_ln2b", [P, CT], F32, isOutput=False)
        W[f"l{l}_qkvw"] = dp(f"l{l}_qkvw", [P, CT, 2, 3 * H * D], BF16, isOutput=False)
        W[f"l{l}_outw"] = dp(f"l{l}_outw", [P, CT, 2, C], BF16, isOutput=False)
        W[f"l{l}_outb"] = dp(f"l{l}_outb", [P, CT], F32, isOutput=False)
        if l < NDENSE:
            W[f"l{l}_ffw1"] = dp(f"l{l}_ffw1", [P, CT, 2, HID], BF16, isOutput=False)
            W[f"l{l}_ffb1"] = dp(f"l{l}_ffb1", [P, HT], F32, isOutput=False)
            W[f"l{l}_ffw2"] = dp(f"l{l}_ffw2", [P, HT, 2, C], BF16, isOutput=False)
            W[f"l{l}_ffb2"] = dp(f"l{l}_ffb2", [P, CT], F32, isOutput=False)
        else:
            W[f"l{l}_gwT"] = dp(f"l{l}_gwT", [P, CT, E], F32, isOutput=False)
            if l < L - 1:   # hi/lo bf16 pairs (3-term compensated matmuls)
                W[f"l{l}_exw1"] = dp(f"l{l}_exw1", [E, P, CT, 2, HID], BF16, isOutput=False)
                W[f"l{l}_exw3"] = dp(f"l{l}_exw3", [E, P, CT, 2, HID], BF16, isOutput=False)
                W[f"l{l}_exw2"] = dp(f"l{l}_exw2", [E, P, HT, 2, C], BF16, isOutput=False)
            else:           # last layer feeds no gate: plain bf16
                W[f"l{l}_exw1"] = dp(f"l{l}_exw1", [E, P, CT, HID], BF16, isOutput=False)
                W[f"l{l}_exw3"] = dp(f"l{l}_exw3", [E, P, CT, HID], BF16, isOutput=False)
                W[f"l{l}_exw2"] = dp(f"l{l}_exw2", [E, P, HT, C], BF16, isOutput=False)
            W[f"l{l}_exb1"] = dp(f"l{l}_exb1", [E, P, HT], F32, isOutput=False)
            W[f"l{l}_exb3"] = dp(f"l{l}_exb3", [E, P, HT], F32, isOutput=False)
            W[f"l{l}_exb2r"] = dp(f"l{l}_exb2r", [E, CT, P], F32, isOutput=False)

    with tile.TileContext(nc) as tc:
        _emit(nc, tc, xt_in, out_d, W)
    nc.compile()
    return nc


def _emit(nc, tc, xt_in, out_d, W):
    from contextlib import ExitStack
    ctx = ExitStack()
    const = ctx.enter_context(tc.tile_pool(name="const", bufs=1))
    big = ctx.enter_context(tc.tile_pool(name="big", bufs=1))
    wp = ctx.enter_context(tc.tile_pool(name="wp", bufs=2))
    ptp = ctx.enter_context(tc.tile_pool(name="ptp", bufs=3))
    sm = ctx.enter_context(tc.tile_pool(name="sm", bufs=1))
    sm2 = ctx.enter_context(tc.tile_pool(name="sm2", bufs=2))
    dram = ctx.enter_context(tc.tile_pool(name="dram", bufs=1, space="DRAM"))
    ps_mm = ctx.enter_context(tc.tile_pool(name="ps_mm", bufs=2, space="PSUM"))
    ps_aux = ctx.enter_context(tc.tile_pool(name="ps_aux", bufs=2, space="PSUM"))
    ps_av = ctx.enter_context(tc.tile_pool(name="ps_av", bufs=2, space="PSUM"))
    ps_st = ctx.enter_context(tc.tile_pool(name="ps_st", bufs=2, space="PSUM"))

    ident = const.tile([P, P], F32)
    make_identity(nc, ident[:])
    ones_c = const.tile([P, 1], F32)      # column of ones (lhsT for col-sums)
    nc.vector.memset(ones_c[:], 1.0)
    ones_r = const.tile([1, P], F32)      # row of ones (lhsT for broadcasts)
    nc.vector.memset(ones_r[:], 1.0)

    # residual, feature-major [P, CT, T]
    resid = const.tile([P, CT, T], F32)
    nc.sync.dma_start(out=resid[:], in_=xt_in[:])

    pid = nc.sync.partition_id()
    pair_base = (pid // 2) * 2 * P        # AG-row offset of my batch pair

    def layer_norm(l, which, dst_tag):
        """LN over the feature (partition x ctile) axis; returns h tile."""
        g = sm.tile([P, CT], F32, tag="lng", name=f"g{l}{which}")
        b = sm.tile([P, CT], F32, tag="lnb", name=f"b{l}{which}")
        nc.sync.dma_start(out=g[:], in_=W[f"l{l}_ln{which}g"][:])
        nc.sync.dma_start(out=b[:], in_=W[f"l{l}_ln{which}b"][:])
        sq = big.tile([P, CT, T], F32, tag="tmp2m", name=f"sq{l}{which}")
        for ct in range(CT):
            nc.vector.tensor_mul(out=sq[:, ct, :], in0=resid[:, ct, :],
                                 in1=resid[:, ct, :])
        ps1 = ps_st.tile([1, T], F32, tag="st", name=f"ps1_{l}{which}")
        ps2 = ps_st.tile([1, T], F32, tag="st", name=f"ps2_{l}{which}")
        for ct in range(CT):
            nc.tensor.matmul(out=ps1[:], lhsT=ones_c[:], rhs=resid[:, ct, :],
                             start=(ct == 0), stop=(ct == CT - 1))
        for ct in range(CT):
            nc.tensor.matmul(out=ps2[:], lhsT=ones_c[:], rhs=sq[:, ct, :],
                             start=(ct == 0), stop=(ct == CT - 1))
        st = sm.tile([1, 6, T], F32, tag="stats", name=f"st{l}{which}")
        mu, msq, varp, rinv, rstd, numu = (st[:, i, :] for i in range(6))
        nc.vector.tensor_scalar_mul(out=mu, in0=ps1[:], scalar1=1.0 / C)
        nc.vector.tensor_scalar_mul(out=msq, in0=ps2[:], scalar1=1.0 / C)
        nc.vector.tensor_mul(out=varp, in0=mu, in1=mu)
        nc.vector.tensor_tensor(out=varp, in0=msq, in1=varp, op=OP.subtract)
        nc.vector.tensor_scalar_add(out=varp, in0=varp, scalar1=EPS)
        sd = st[:, 1, :]  # reuse msq slot
        nc.scalar.activation(out=sd, in_=varp, func=AF.Sqrt)
        nc.vector.reciprocal(out=rinv, in_=sd)
        # one Newton step: r = rinv*(1.5 - 0.5*varp*rinv^2)
        nc.vector.tensor_mul(out=sd, in0=rinv, in1=rinv)
        nc.vector.tensor_mul(out=sd, in0=sd, in1=varp)
        nc.vector.tensor_scalar(out=sd, in0=sd, scalar1=-0.5, scalar2=1.5,
                                op0=OP.mult, op1=OP.add)
        nc.vector.tensor_mul(out=rstd, in0=rinv, in1=sd)
        nc.vector.tensor_mul(out=numu, in0=mu, in1=rstd)
        nc.vector.tensor_scalar_mul(out=numu, in0=numu, scalar1=-1.0)
        psR = ps_aux.tile([P, T], F32, tag="aux", name=f"psR{l}{which}")
        psM = ps_aux.tile([P, T], F32, tag="aux", name=f"psM{l}{which}")
        nc.tensor.matmul(out=psR[:], lhsT=ones_r[:], rhs=rstd, start=True, stop=True)
        nc.tensor.matmul(out=psM[:], lhsT=ones_r[:], rhs=numu, start=True, stop=True)
        h = big.tile([P, CT, T], F32, tag=dst_tag, name=f"h{l}{which}")
        for ct in range(CT):
            nc.vector.tensor_tensor(out=h[:, ct, :], in0=resid[:, ct, :],
                                    in1=psR[:], op=OP.mult)
            nc.vector.tensor_tensor(out=h[:, ct, :], in0=h[:, ct, :],
                                    in1=psM[:], op=OP.add)
            nc.vector.tensor_scalar(out=h[:, ct, :], in0=h[:, ct, :],
                                    scalar1=g[:, ct:ct + 1], scalar2=b[:, ct:ct + 1],
                                    op0=OP.mult, op1=OP.add)
        return h

    def split_pair(srcf32, kt, tag, nm):
        """f32 [P, kt, T] -> bf16 hi/lo pair [P, 2, kt, T]"""
        hp = big.tile([P, 2, kt, T], BF16, tag=tag, name=f"sp{nm}")
        for k in range(kt):
            nc.vector.tensor_copy(out=hp[:, 0, k, :], in_=srcf32[:, k, :])
            nc.vector.tensor_tensor(out=hp[:, 1, k, :], in0=srcf32[:, k, :],
                                    in1=hp[:, 0, k, :], op=OP.subtract)
        return hp

    def matmul_block(dst, dst_slice_fn, w_dram, hp, kt, n_cols, l, nm,
                     act=None, bias=None, chunk_cols=512):
        """dst[.., n] = act(w.T @ h + bias); w_dram [P, kt, 2, n_cols] bf16
        hi/lo pairs, hp [P, 2, kt, T] bf16 hi/lo; 3-term compensated."""
        for c0 in range(0, n_cols, chunk_cols):
            cw = min(chunk_cols, n_cols - c0)
            wc = wp.tile([P, kt, 2, chunk_cols], BF16, tag="w", name=f"w{nm}_{c0}")
            nc.sync.dma_start(out=wc[:, :, :, :cw], in_=w_dram[:, :, :, c0:c0 + cw])
            for ni in range(0, cw, P):
                psx = ps_mm.tile([P, T], F32, tag="mm", name=f"ps{nm}_{c0}_{ni}")
                for k in range(kt):
                    wh, wl = wc[:, k, 0, ni:ni + P], wc[:, k, 1, ni:ni + P]
                    nc.tensor.matmul(out=psx[:], lhsT=wh, rhs=hp[:, 0, k, :],
                                     start=(k == 0), stop=False)
                    nc.tensor.matmul(out=psx[:], lhsT=wh, rhs=hp[:, 1, k, :],
                                     start=False, stop=False)
                    nc.tensor.matmul(out=psx[:], lhsT=wl, rhs=hp[:, 0, k, :],
                                     start=False, stop=(k == kt - 1))
                n_idx = (c0 + ni) // P
                dslice = dst_slice_fn(dst, n_idx)
                if act is not None:
                    bb = bias[:, n_idx:n_idx + 1] if bias is not None else 0.0
                    nc.scalar.activation(out=dslice, in_=psx[:], func=act, bias=bb)
                elif bias is not None:
                    nc.vector.tensor_scalar_add(out=dslice, in0=psx[:],
                                                scalar1=bias[:, n_idx:n_idx + 1])
                else:
                    nc.vector.tensor_copy(out=dslice, in_=psx[:])

    for l in range(L):
        # ---- LN1 + attention ----
        h = layer_norm(l, 1, "h")
        h1p = split_pair(h, CT, "tmp2m", f"h1p{l}")
        qkv_d = W[f"l{l}_qkvw"]
        # keys stored MY-HALF-FIRST (softmax is key-permutation invariant):
        # kTf[:, :, 0:512] = my keys (written straight from the k-matmuls),
        # kTf[:, :, 512:1024] = partner's (arrives via AllGather readback).
        kTf = big.tile([P, CT, 1024], F32, tag="big4a", name=f"kTf{l}")
        vaug = big.tile([P, 8, H, 65], F32, tag="big4b", name=f"vaug{l}")
        nc.vector.memset(vaug[:, :, :, 64:65], 1.0)
        matmul_block(kTf, lambda d, n: d[:, n, 0:512], qkv_d[:, :, :, 1024:2048],
                     h1p, CT, 1024, l, f"k{l}")
        cin = dram.tile([P, 8192], F32, tag="cin", name=f"cin{l}")
        cout = dram.tile([NC_ * P, 8192], F32, tag="cout", name=f"cout{l}")
        nc.gpsimd.dma_start(out=cin[:, 0:4096].rearrange("p (c t) -> p c t", c=CT),
                            in_=kTf[:, :, 0:512])
        # v token-major into vaug chunks 0..3; lhsT = h token-chunks (hi/lo)
        for nv in range(2):
            wc = wp.tile([P, CT, 2, 512], BF16, tag="w", name=f"wv{l}_{nv}")
            nc.sync.dma_start(out=wc[:],
                              in_=qkv_d[:, :, :, 2048 + nv * 512: 2048 + (nv + 1) * 512])
            for m in range(4):
                psv = ps_mm.tile([P, T], F32, tag="mm", name=f"psv{l}_{m}_{nv}")
                for k in range(CT):
                    lh = h1p[:, 0, k, m * P:(m + 1) * P]
                    ll = h1p[:, 1, k, m * P:(m + 1) * P]
                    nc.tensor.matmul(out=psv[:], lhsT=lh, rhs=wc[:, k, 0, :],
                                     start=(k == 0), stop=False)
                    nc.tensor.matmul(out=psv[:], lhsT=ll, rhs=wc[:, k, 0, :],
                                     start=False, stop=False)
                    nc.tensor.matmul(out=psv[:], lhsT=lh, rhs=wc[:, k, 1, :],
                                     start=False, stop=(k == CT - 1))
                nc.vector.tensor_copy(
                    out=vaug[:, m, nv * 8:(nv + 1) * 8, 0:64], in_=psv[:])
        # v bounce, then the AllGather; q-matmuls + my-half attention overlap
        nc.gpsimd.dma_start(
            out=cin[:, 4096:8192].rearrange("p (c h d) -> p c h d", c=4, h=H),
            in_=vaug[:, 0:4, :, 0:64])
        nc.gpsimd.collective_compute(
            "AllGather", OP.bypass, replica_groups=[list(range(NC_))],
            ins=[cin.opt()], outs=[cout.opt()])
        qT = big.tile([P, CT, T], F32, tag="qT", name=f"qT{l}")
        matmul_block(qT, lambda d, n: d[:, n, :], qkv_d[:, :, :, 0:1024], h1p,
                     CT, 1024, l, f"q{l}")
        # my-half attention for heads 0..7 (no partner data needed); the psA
        # partial is copied out so the psum bank frees immediately
        av_pack = big.tile([65, 8, T], F32, tag="h", name=f"avp{l}")
        for hd in range(8):
            hp, half = hd // 2, hd % 2
            rows = slice(half * 64, half * 64 + 64)
            psA = ps_av.tile([65, T], F32, tag="av", name=f"psA{l}_{hd}a")
            for kc in range(4):
                psS = ps_aux.tile([P, T], F32, tag="aux", name=f"psS{l}_{hd}_{kc}")
                nc.tensor.matmul(out=psS[:], lhsT=kTf[rows, hp, kc * P:(kc + 1) * P],
                                 rhs=qT[rows, hp, :], start=True, stop=True)
                pt = ptp.tile([P, T], F32, tag="pt", name=f"pt{l}_{hd}_{kc}")
                nc.scalar.activation(out=pt[:], in_=psS[:], func=AF.Exp, scale=0.125)
                nc.tensor.matmul(out=psA[:], lhsT=vaug[:, kc, hd, :], rhs=pt[:],
                                 start=(kc == 0), stop=(kc == 3))
            nc.vector.tensor_copy(out=av_pack[:, hd, :], in_=psA[:])
        # partner readback: row (pid^1) of the AllGather output
        parity = pid - (pid // 2) * 2
        partner = nc.s_assert_within(pid + 1 - parity * 2, 0, NC_ - 1, skip_runtime_assert=True)
        srcp = cout[bass.ds(partner * P, P), :]
        nc.sync.dma_start(
            out=kTf[:, :, 512:1024],
            in_=srcp[:, 0:4096].rearrange("p (c t) -> p c t", c=CT))
        nc.sync.dma_start(
            out=vaug[:, 4:8, :, 0:64],
            in_=srcp[:, 4096:8192].rearrange("p (c h d) -> p c h d", c=4, h=H))
        attT = big.tile([P, CT, T], F32, tag="tmp2m", name=f"attT{l}")
        for hd in range(H):
            hp, half = hd // 2, hd % 2
            rows = slice(half * 64, half * 64 + 64)
            two_pass = hd < 8
            kc0 = 4 if two_pass else 0
            psA = ps_av.tile([65, T], F32, tag="av", name=f"psA{l}_{hd}b")
            for kc in range(kc0, 8):
                psS = ps_aux.tile([P, T], F32, tag="aux", name=f"psS{l}_{hd}_{kc}b")
                nc.tensor.matmul(out=psS[:], lhsT=kTf[rows, hp, kc * P:(kc + 1) * P],
                                 rhs=qT[rows, hp, :], start=True, stop=True)
                pt = ptp.tile([P, T], F32, tag="pt", name=f"pt{l}_{hd}_{kc}b")
                nc.scalar.activation(out=pt[:], in_=psS[:], func=AF.Exp, scale=0.125)
                nc.tensor.matmul(out=psA[:], lhsT=vaug[:, kc, hd, :], rhs=pt[:],
                                 start=(kc == kc0), stop=(kc == 7))
            av = sm2.tile([65, T], F32, tag="avs", name=f"av{l}_{hd}")
            if two_pass:
                nc.vector.tensor_tensor(out=av[:], in0=av_pack[:, hd, :],
                                        in1=psA[:], op=OP.add)
            else:
                nc.vector.tensor_copy(out=av[:], in_=psA[:])
            rec = sm2.tile([1, T], F32, tag="rec", name=f"rec{l}_{hd}")
            nc.vector.reciprocal(out=rec[:], in_=av[64:65, :])
            psB = ps_aux.tile([64, T], F32, tag="aux", name=f"psB{l}_{hd}")
            nc.tensor.matmul(out=psB[:], lhsT=ones_r[:, 0:64], rhs=rec[:],
                             start=True, stop=True)
            nc.vector.tensor_tensor(out=attT[rows, hp, :], in0=av[0:64, :],
                                    in1=psB[:], op=OP.mult)
        # out-projection + residual add
        outb = sm.tile([P, CT], F32, tag="lnb2", name=f"outb{l}")
        nc.sync.dma_start(out=outb[:], in_=W[f"l{l}_outb"][:])
        attTp = split_pair(attT, CT, "h", f"attTp{l}")
        ow_d = W[f"l{l}_outw"]
        for c0 in (0, 512):
            wc = wp.tile([P, CT, 2, 512], BF16, tag="w", name=f"wo{l}_{c0}")
            nc.sync.dma_start(out=wc[:], in_=ow_d[:, :, :, c0:c0 + 512])
            for ni in range(4):
                ct = (c0 + ni * P) // P
                psx = ps_mm.tile([P, T], F32, tag="mm", name=f"pso{l}_{ct}")
                for k in range(CT):
                    wh, wl = wc[:, k, 0, ni * P:(ni + 1) * P], wc[:, k, 1, ni * P:(ni + 1) * P]
                    nc.tensor.matmul(out=psx[:], lhsT=wh, rhs=attTp[:, 0, k, :],
                                     start=(k == 0), stop=False)
                    nc.tensor.matmul(out=psx[:], lhsT=wh, rhs=attTp[:, 1, k, :],
                                     start=False, stop=False)
                    nc.tensor.matmul(out=psx[:], lhsT=wl, rhs=attTp[:, 0, k, :],
                                     start=False, stop=(k == CT - 1))
                tb = sm2.tile([P, T], F32, tag="projtmp", name=f"tb{l}_{ct}")
                nc.vector.tensor_scalar_add(out=tb[:], in0=psx[:],
                                            scalar1=outb[:, ct:ct + 1])
                nc.vector.tensor_tensor(out=resid[:, ct, :], in0=resid[:, ct, :],
                                        in1=tb[:], op=OP.add)

        # ---- LN2 + FFN/MoE ----
        h2 = layer_norm(l, 2, "h")
        if l < NDENSE:
            ffb1 = sm.tile([P, HT], F32, tag="lnb2", name=f"ffb1{l}")
            nc.sync.dma_start(out=ffb1[:], in_=W[f"l{l}_ffb1"][:])
            h2p = split_pair(h2, CT, "tmp2m", f"h2pd{l}")
            f1 = big.tile([P, HT, T], F32, tag="big4a", name=f"f1{l}")
            matmul_block(f1, lambda d, n: d[:, n, :], W[f"l{l}_ffw1"], h2p, CT,
                         HID, l, f"ff1{l}", act=AF.Gelu, bias=ffb1)
            ffb2 = sm.tile([P, CT], F32, tag="lnb2", name=f"ffb2{l}")
            nc.sync.dma_start(out=ffb2[:], in_=W[f"l{l}_ffb2"][:])
            f1p = split_pair(f1, HT, "big4b", f"f1p{l}")
            f2w = W[f"l{l}_ffw2"]
            for ct in range(CT):
                psx = ps_mm.tile([P, T], F32, tag="mm", name=f"psf2{l}_{ct}")
                for kh in range(2):
                    wc = wp.tile([P, CT, 2, P], BF16, tag="w", name=f"wf2{l}_{ct}_{kh}")
                    nc.sync.dma_start(
                        out=wc[:],
                        in_=f2w[:, kh * CT:(kh + 1) * CT, :, ct * P:(ct + 1) * P])
                    for k in range(CT):
                        kk = kh * CT + k
                        wh, wl = wc[:, k, 0, :], wc[:, k, 1, :]
                        nc.tensor.matmul(out=psx[:], lhsT=wh, rhs=f1p[:, 0, kk, :],
                                         start=(kk == 0), stop=False)
                        nc.tensor.matmul(out=psx[:], lhsT=wh, rhs=f1p[:, 1, kk, :],
                                         start=False, stop=False)
                        nc.tensor.matmul(out=psx[:], lhsT=wl, rhs=f1p[:, 0, kk, :],
                                         start=False, stop=(kk == HT - 1))
                tb = sm2.tile([P, T], F32, tag="projtmp", name=f"tf{l}_{ct}")
                nc.vector.tensor_scalar_add(out=tb[:], in0=psx[:],
                                            scalar1=ffb2[:, ct:ct + 1])
                nc.vector.tensor_tensor(out=resid[:, ct, :], in0=resid[:, ct, :],
                                        in1=tb[:], op=OP.add)
        else:
            _moe(nc, tc, l, W, h2, resid, ident, ones_r, big, wp, sm, sm2,
                 ps_mm, ps_aux, ps_st)

    nc.sync.dma_start(out=out_d[:], in_=resid[:])
    ctx.close()


def _moe(nc, tc, l, W, h2, resid, ident, ones_r, big, wp, sm, sm2, ps_mm, ps_aux, ps_st):
    # gate logits [E, T] feature-major
    gw = sm.tile([P, CT, E], F32, tag="gw", name=f"gw{l}")
    nc.sync.dma_start(out=gw[:], in_=W[f"l{l}_gwT"][:])
    psg = ps_st.tile([E, T], F32, tag="st", name=f"psg{l}")
    for k in range(CT):
        nc.tensor.matmul(out=psg[:], lhsT=gw[:, k, :], rhs=h2[:, k, :],
                         start=(k == 0), stop=(k == CT - 1))
    lg = sm.tile([E, T], F32, tag="lg", name=f"lg{l}")
    nc.vector.tensor_copy(out=lg[:], in_=psg[:])
    # transpose to token-major [128, 4, E]
    lgT = sm.tile([P, 4, E], F32, tag="lgT", name=f"lgT{l}")
    for j in range(4):
        pst = ps_st.tile([P, E], F32, tag="st", name=f"pst{l}_{j}")
        nc.tensor.transpose(out=pst[:], in_=lg[:, j * P:(j + 1) * P],
                            identity=ident[0:E, 0:E])
        nc.vector.tensor_copy(out=lgT[:, j, :], in_=pst[:])
    # top-2 mask + softmax (max-subtracted, matching reference)
    wk = sm.tile([P, 4, 6, E], F32, tag="wk", name=f"wk{l}")
    m1 = sm.tile([P, 4, 4], F32, tag="m1", name=f"m1{l}")
    for j in range(4):
        nc.vector.tensor_reduce(out=m1[:, j, 0:1], in_=lgT[:, j, :],
                                axis=mybir.AxisListType.X, op=OP.max)
        # eq-mask of the max, knock it out, then second max
        nc.vector.tensor_scalar(out=wk[:, j, 0, :], in0=lgT[:, j, :],
                                scalar1=m1[:, j, 0:1], scalar2=None,
                                op0=OP.is_equal)
        nc.vector.tensor_scalar_mul(out=wk[:, j, 1, :], in0=wk[:, j, 0, :],
                                    scalar1=1e30)
        nc.vector.tensor_tensor(out=wk[:, j, 1, :], in0=lgT[:, j, :],
                                in1=wk[:, j, 1, :], op=OP.subtract)
        nc.vector.tensor_reduce(out=m1[:, j, 1:2], in_=wk[:, j, 1, :],
                                axis=mybir.AxisListType.X, op=OP.max)
        nc.vector.tensor_scalar(out=wk[:, j, 2, :], in0=lgT[:, j, :],
                                scalar1=m1[:, j, 1:2], scalar2=None,
                                op0=OP.is_ge)
        # softmax exp(x - max)
        nc.vector.tensor_scalar_mul(out=m1[:, j, 2:3], in0=m1[:, j, 0:1],
                                    scalar1=-1.0)
        nc.scalar.activation(out=wk[:, j, 3, :], in_=lgT[:, j, :], func=AF.Exp,
                             bias=m1[:, j, 2:3])
        nc.vector.tensor_reduce(out=m1[:, j, 3:4], in_=wk[:, j, 3, :],
                                axis=mybir.AxisListType.X, op=OP.add)
        nc.vector.reciprocal(out=m1[:, j, 3:4], in_=m1[:, j, 3:4])
        nc.vector.tensor_mul(out=wk[:, j, 4, :], in0=wk[:, j, 3, :],
                             in1=wk[:, j, 2, :])
        nc.vector.tensor_scalar_mul(out=wk[:, j, 5, :], in0=wk[:, j, 4, :],
                                    scalar1=m1[:, j, 3:4])
    # weT [E, T] feature-major combine weights
    weT = sm.tile([E, T], F32, tag="lg2", name=f"weT{l}")
    for j in range(4):
        pst = ps_st.tile([E, P], F32, tag="st", name=f"psu{l}_{j}")
        nc.tensor.transpose(out=pst[:], in_=wk[:, j, 5, :], identity=ident[:])
        nc.vector.tensor_copy(out=weT[:, j * P:(j + 1) * P], in_=pst[:])

    moe_out = big.tile([P, CT, T], F32, tag="qT", name=f"moeout{l}")
    b2mode = (l < L - 1)
    # split h2 into bf16 hi (+ lo for the compensated layers)
    h2p = big.tile([P, 2, CT, T], BF16, tag="tmp2m", name=f"h2p{l}")
    for ct in range(CT):
        nc.vector.tensor_copy(out=h2p[:, 0, ct, :], in_=h2[:, ct, :])
        if b2mode:
            nc.vector.tensor_tensor(out=h2p[:, 1, ct, :], in0=h2[:, ct, :],
                                    in1=h2p[:, 0, ct, :], op=OP.subtract)

    def mm_terms(psx, wh, wl, rh, rl, k, kt):
        n_terms = 3 if b2mode else 1
        first = (k == 0)
        last = (k == kt - 1)
        nc.tensor.matmul(out=psx[:], lhsT=wh, rhs=rh, start=first,
                         stop=(last and n_terms == 1))
        if b2mode:
            nc.tensor.matmul(out=psx[:], lhsT=wh, rhs=rl, start=False, stop=False)
            nc.tensor.matmul(out=psx[:], lhsT=wl, rhs=rh, start=False, stop=last)

    for e in range(E):
        b1 = sm.tile([P, HT], F32, tag="lnb2", name=f"exb1{l}_{e}")
        b3 = sm.tile([P, HT], F32, tag="lnb3", name=f"exb3{l}_{e}")
        nc.sync.dma_start(out=b1[:], in_=W[f"l{l}_exb1"][e])
        nc.sync.dma_start(out=b3[:], in_=W[f"l{l}_exb3"][e])
        g1 = big.tile([P, HT, T], F32, tag="big4a", name=f"g1_{l}_{e}")
        g3 = big.tile([P, HT, T], F32, tag="big4b", name=f"g3_{l}_{e}")
        w1d, w3d, w2d = (W[f"l{l}_exw1"][e], W[f"l{l}_exw3"][e], W[f"l{l}_exw2"][e])
        for c0 in range(0, HID, 512):
            for (wd, gg, bb, acts) in ((w1d, g1, b1, AF.Silu), (w3d, g3, b3, None)):
                if b2mode:
                    wc = wp.tile([P, CT, 2, 512], BF16, tag="w",
                                 name=f"we{l}_{e}_{c0}_{acts}")
                    nc.sync.dma_start(out=wc[:], in_=wd[:, :, :, c0:c0 + 512])
                else:
                    wc = wp.tile([P, CT, 512], BF16, tag="w",
                                 name=f"we{l}_{e}_{c0}_{acts}")
                    nc.sync.dma_start(out=wc[:], in_=wd[:, :, c0:c0 + 512])
                for ni in range(4):
                    nt = (c0 + ni * P) // P
                    psx = ps_mm.tile([P, T], F32, tag="mm",
                                     name=f"pse{l}_{e}_{nt}_{acts}")
                    for k in range(CT):
                        if b2mode:
                            wh, wl = wc[:, k, 0, ni * P:(ni + 1) * P], wc[:, k, 1, ni * P:(ni + 1) * P]
                        else:
                            wh = wl = wc[:, k, ni * P:(ni + 1) * P]
                        mm_terms(psx, wh, wl, h2p[:, 0, k, :], h2p[:, 1, k, :], k, CT)
                    if acts is not None:
                        nc.scalar.activation(out=gg[:, nt, :], in_=psx[:],
                                             func=acts, bias=bb[:, nt:nt + 1])
                    else:
                        nc.vector.tensor_scalar_add(out=gg[:, nt, :], in0=psx[:],
                                                    scalar1=bb[:, nt:nt + 1])
        # we_e broadcast (K=1 matmul; row DMA'd to partition 0 first)
        werow = sm2.tile([1, T], F32, tag="werow", name=f"werow{l}_{e}")
        nc.sync.dma_start(out=werow[:], in_=weT[e:e + 1, :])
        psW = ps_aux.tile([P, T], F32, tag="aux", name=f"psW{l}_{e}")
        nc.tensor.matmul(out=psW[:], lhsT=ones_r[:], rhs=werow[:],
                         start=True, stop=True)
        # ge = silu(g1)*g3*we -> f32 in g3, then bf16 hi/lo for the w2 matmul
        for k in range(HT):
            nc.vector.tensor_tensor(out=g1[:, k, :], in0=g1[:, k, :],
                                    in1=g3[:, k, :], op=OP.mult)
            nc.vector.tensor_tensor(out=g3[:, k, :], in0=g1[:, k, :],
                                    in1=psW[:], op=OP.mult)
        gehl = big.tile([P, 2, HT, T], BF16, tag="big4a", name=f"gehl{l}_{e}")
        for k in range(HT):
            nc.vector.tensor_copy(out=gehl[:, 0, k, :], in_=g3[:, k, :])
            if b2mode:
                nc.vector.tensor_tensor(out=gehl[:, 1, k, :], in0=g3[:, k, :],
                                        in1=gehl[:, 0, k, :], op=OP.subtract)
        # w2: [HID -> C], accumulate over experts in moe_out
        for ct in range(CT):
            psx = ps_mm.tile([P, T], F32, tag="mm", name=f"ps2{l}_{e}_{ct}")
            for kh in range(2):
                if b2mode:
                    wc = wp.tile([P, CT, 2, P], BF16, tag="w",
                                 name=f"w2{l}_{e}_{ct}_{kh}")
                    nc.sync.dma_start(
                        out=wc[:],
                        in_=w2d[:, kh * CT:(kh + 1) * CT, :, ct * P:(ct + 1) * P])
                else:
                    wc = wp.tile([P, CT, P], BF16, tag="w",
                                 name=f"w2{l}_{e}_{ct}_{kh}")
                    nc.sync.dma_start(
                        out=wc[:],
                        in_=w2d[:, kh * CT:(kh + 1) * CT, ct * P:(ct + 1) * P])
                for k in range(CT):
                    if b2mode:
                        wh, wl = wc[:, k, 0, :], wc[:, k, 1, :]
                    else:
                        wh = wl = wc[:, k, :]
                    kk = kh * CT + k
                    mm_terms(psx, wh, wl, gehl[:, 0, kk, :], gehl[:, 1, kk, :],
                             kk, HT)
            if e == 0:
                nc.vector.tensor_copy(out=moe_out[:, ct, :], in_=psx[:])
            else:
                nc.vector.tensor_tensor(out=moe_out[:, ct, :],
                                        in0=moe_out[:, ct, :], in1=psx[:],
                                        op=OP.add)
    # bias contribution: sum_e we_e * b2_e  == exb2r.T @ weT  (contraction E)
    b2r = sm.tile([E, CT, P], F32, tag="b2r", name=f"b2r{l}")
    nc.sync.dma_start(out=b2r[:], in_=W[f"l{l}_exb2r"][:])
    for ct in range(CT):
        psx = ps_aux.tile([P, T], F32, tag="aux", name=f"psb2{l}_{ct}")
        nc.tensor.matmul(out=psx[:], lhsT=b2r[:, ct, :], rhs=weT[:],
                         start=True, stop=True)
        nc.vector.tensor_tensor(out=moe_out[:, ct, :], in0=moe_out[:, ct, :],
                                in1=psx[:], op=OP.add)
        nc.vector.tensor_tensor(out=resid[:, ct, :], in0=resid[:, ct, :],
                                in1=moe_out[:, ct, :], op=OP.add)


def _stage_weights(params):
    fl = {}
    for l, p in enumerate(params):
        g = lambda k: np.ascontiguousarray(np.asarray(p[k], dtype=np.float32))
        fl[f"l{l}_ln1g"] = g("ln1_g").reshape(CT, P).T.copy()
        fl[f"l{l}_ln1b"] = g("ln1_b").reshape(CT, P).T.copy()
        fl[f"l{l}_ln2g"] = g("ln2_g").reshape(CT, P).T.copy()
        fl[f"l{l}_ln2b"] = g("ln2_b").reshape(CT, P).T.copy()
        import ml_dtypes
        _bf = ml_dtypes.bfloat16
        def split2(w):  # [P, kt, n] f32 -> [P, kt, 2, n] bf16 hi/lo
            wh = w.astype(_bf)
            wl = (w - wh.astype(np.float32)).astype(_bf)
            return np.ascontiguousarray(np.stack([wh, wl], axis=2))
        fl[f"l{l}_qkvw"] = split2(g("qkv_w").reshape(CT, P, 3 * H * D).transpose(1, 0, 2))
        fl[f"l{l}_outw"] = split2(g("out_w").reshape(CT, P, C).transpose(1, 0, 2))
        fl[f"l{l}_outb"] = g("out_b").reshape(CT, P).T.copy()
        if "ff_w1" in p:
            fl[f"l{l}_ffw1"] = split2(g("ff_w1").reshape(CT, P, HID).transpose(1, 0, 2))
            fl[f"l{l}_ffb1"] = g("ff_b1").reshape(HT, P).T.copy()
            fl[f"l{l}_ffw2"] = split2(g("ff_w2").reshape(HT, P, C).transpose(1, 0, 2))
            fl[f"l{l}_ffb2"] = g("ff_b2").reshape(CT, P).T.copy()
        else:
            fl[f"l{l}_gwT"] = np.ascontiguousarray(g("gate_w").T).reshape(CT, P, E).transpose(1, 0, 2).copy()
            import ml_dtypes
            bf = ml_dtypes.bfloat16
            w1 = g("ex_w1").reshape(E, CT, P, HID).transpose(0, 2, 1, 3)
            w3 = g("ex_w3").reshape(E, CT, P, HID).transpose(0, 2, 1, 3)
            w2 = g("ex_w2").reshape(E, HT, P, C).transpose(0, 2, 1, 3)
            if l < L - 1:
                def split(w):
                    wh = w.astype(bf)
                    wl = (w - wh.astype(np.float32)).astype(bf)
                    return np.ascontiguousarray(np.stack([wh, wl], axis=3))
                fl[f"l{l}_exw1"] = split(w1)
                fl[f"l{l}_exw3"] = split(w3)
                fl[f"l{l}_exw2"] = split(w2)
            else:
                fl[f"l{l}_exw1"] = np.ascontiguousarray(w1.astype(bf))
                fl[f"l{l}_exw3"] = np.ascontiguousarray(w3.astype(bf))
                fl[f"l{l}_exw2"] = np.ascontiguousarray(w2.astype(bf))
            fl[f"l{l}_exb1"] = g("ex_b1").reshape(E, HT, P).transpose(0, 2, 1).copy()
            fl[f"l{l}_exb3"] = g("ex_b3").reshape(E, HT, P).transpose(0, 2, 1).copy()
            fl[f"l{l}_exb2r"] = g("ex_b2").reshape(E, CT, P).copy()
    return fl


_NC_CACHE = None


def kernel(x, params):
    global _NC_CACHE, LAST_RESULT
    x = np.asarray(x, dtype=np.float32)
    staged = _stage_weights(params)
    if _NC_CACHE is None:
        _NC_CACHE = _build()
    nc = _NC_CACHE
    xr = x.reshape(B, 2, T, C)
    in_maps = []
    for c in range(NC_):
        shard = xr[c // 2, c % 2]                      # [T, C]
        xt = np.ascontiguousarray(shard.T.reshape(CT, P, T).transpose(1, 0, 2))
        m = {"xt": xt}
        m.update(staged)
        in_maps.append(m)
    res = run_bass_kernel_spmd(nc, in_maps, list(range(NC_)),
                               trace=bool(os.environ.get("BASS_TRACE")))
    LAST_RESULT = res
    out = np.empty((B, 2, T, C), np.float32)
    for c in range(NC_):
        ot = res.results[c]["out_t"]                   # [P, CT, T]
        out[c // 2, c % 2] = ot.transpose(1, 0, 2).reshape(C, T).T
    return out.reshape(B, N, C)
